# revision 36
# baseline (speedup 1.0000x reference)
"""Trainium2 Bass kernel for nn_EpipolarWarpOperator (B=8, C=320, H=W=64, S=3).

Sharding: spatial — every core computes an 8-row horizontal strip of all 8
batch outputs (the 3x3 conv is strip-separable; epipolar sampling per strip
with a 1-row halo).

Host analysis classifies each batch by its epipolar sampling map:

* pattern batches: the per-pixel bilinear sampling signature map has few
  distinct values, so the sampled image is piecewise constant and each output
  column is one of npat distinct "pattern" columns. The device computes
  R[k, tap, pi] from a host-gathered slab via tiny matmuls (kc=2 taps stored
  pairwise over the partition dim), T^T[pi, ch] = sum R_tap^T W_tap (23
  matmuls), then expands out = T^T.T @ E with a per-core 0/1 selection
  matrix E, bias+relu fused into the PSUM drains.

* dense batches: the whole image touches only a few hundred distinct bilinear
  corner pixels; the host gathers them into per-window slabs [128, C] plus
  window sampling matrices (3 windows: 4+4+2 rows covering the strip plus a
  1-row conv halo); the device runs swapped-operand matmuls accumulating the
  slabs into PSUM, then a 24-matmul-per-mc 3x3 conv over the 10-row sampled
  plane (kc=2 taps paired through a +1-row shifted duplicate) with bias+relu
  on the drain.
"""

import numpy as np

B, C, H, W = 8, 320, 64, 64
SN = 3
HW = H * W
STRIP = 8             # output rows per core
NCORE = 8
MB = [(0, 128), (128, 128), (256, 64)]   # output/input channel chunking
PI_MAX = 126          # total pattern budget
NPAT_MAX = 56         # per-batch pattern cap
NSIG_MAX = 64
SW = STRIP * W        # 512 px per strip
WINS = ((-1, 3), (2, 3), (5, 4))   # dense sampling windows (row0, nrows)

# ---------------------------------------------------------------- host: geometry


def _rodrigues_np(rv):
    theta = np.sqrt((rv * rv).sum())
    r = rv / max(theta, 1e-12)
    I = np.eye(3, dtype=np.float32)
    K = np.array([[0, -r[2], r[1]], [r[2], 0, -r[0]], [-r[1], r[0], 0]],
                 dtype=np.float32)
    R = np.cos(theta) * I + (1 - np.cos(theta)) * np.outer(r, r) + np.sin(theta) * K
    return I if theta < 1e-6 else R


def fundamental_np(Ks, Kt, ps, pt):
    Fs = []
    for b in range(Ks.shape[0]):
        Rs = _rodrigues_np(ps[b, :3].astype(np.float32))
        Rt = _rodrigues_np(pt[b, :3].astype(np.float32))
        ts_, tt_ = ps[b, 3:].astype(np.float32), pt[b, 3:].astype(np.float32)
        R_rel = Rs @ Rt.T
        t_rel = ts_ - R_rel @ tt_
        z = np.float32(0)
        skew = np.array([[z, -t_rel[2], t_rel[1]],
                         [t_rel[2], z, -t_rel[0]],
                         [-t_rel[1], t_rel[0], z]], dtype=np.float32)
        E = skew @ R_rel
        inv_Ks = np.linalg.inv(Ks[b].astype(np.float32))
        inv_Kt = np.linalg.inv(Kt[b].astype(np.float32))
        Fs.append(inv_Kt.T @ E @ inv_Ks)
    return np.stack(Fs).astype(np.float32)


def geometry(F):
    k = np.arange(HW)
    px = (k % W).astype(np.float32)
    py = (k // W).astype(np.float32)
    P = np.stack([px, py, np.ones_like(px)])
    lines = F.T.astype(np.float32) @ P
    a, b_, c = lines[0], lines[1], lines[2]
    W1, H1 = np.float32(W - 1), np.float32(H - 1)
    EPS = np.float32(1e-10)
    x1 = np.clip(-c / (a + EPS), 0.0, W1)
    x2 = np.clip(-(b_ * H1 + c) / (a + EPS), 0.0, W1)
    y1 = np.clip(-c / (b_ + EPS), 0.0, H1)
    y2 = np.clip(-(a * W1 + c) / (b_ + EPS), 0.0, H1)
    t = np.linspace(0.0, 1.0, SN, dtype=np.float32)
    sx = x1[:, None] * (1 - t) + x2[:, None] * t
    sy = y1[:, None] * (1 - t) + y2[:, None] * t
    x0 = np.floor(sx)
    y0 = np.floor(sy)
    wx = (sx - x0).astype(np.float32)
    wy = (sy - y0).astype(np.float32)
    x0i = np.clip(x0, 0, W - 1).astype(np.int32)
    y0i = np.clip(y0, 0, H - 1).astype(np.int32)
    return x0i, y0i, wx, wy


def _corners(geo, p, s):
    """4 bilinear (source pixel row, weight/3) pairs for pixel p, sample s."""
    x0i, y0i, wx, wy = geo
    y0 = int(y0i[p, s]); x0 = int(x0i[p, s])
    x1 = min(x0 + 1, W - 1); y1 = min(y0 + 1, H - 1)
    wxx = np.float32(wx[p, s]); wyy = np.float32(wy[p, s])
    third = np.float32(1.0 / 3.0)
    out = {}
    for ry, rx, ww in ((y0, x0, (1 - wxx) * (1 - wyy)),
                       (y0, x1, wxx * (1 - wyy)),
                       (y1, x0, (1 - wxx) * wyy),
                       (y1, x1, wxx * wyy)):
        rr = ry * W + rx
        out[rr] = out.get(rr, np.float32(0)) + ww * third
    return out


# ------------------------------------------------------------- host: classify


def classify(geo):
    x0i, y0i, wx, wy = geo
    key = np.concatenate([
        y0i.astype(np.float32), x0i.astype(np.float32), wx, wy], axis=1)
    kview = np.ascontiguousarray(key).view([('', key.dtype)] * key.shape[1]).ravel()
    uniq, first, inv = np.unique(kview, return_index=True, return_inverse=True)
    if len(uniq) > NSIG_MAX:
        return None
    tables = []
    for si in range(len(uniq)):
        p = int(first[si])
        contrib = {}
        for s in range(SN):
            for rr, ww in _corners(geo, p, s).items():
                contrib[rr] = contrib.get(rr, np.float32(0)) + ww
        tables.append(sorted(contrib.items()))
    return inv.astype(np.int32), tables


def patterns(siginv):
    """3x3 neighborhood patterns of the signature map (border = -1)."""
    simg = siginv.reshape(H, W)
    pad = np.pad(simg, 1, constant_values=-1)
    neigh = np.stack([pad[dy:dy + H, dx:dx + W].ravel()
                      for dy in range(3) for dx in range(3)], axis=1)
    uniq, pinv = np.unique(neigh, axis=0, return_inverse=True)
    return pinv.astype(np.int32), uniq


# ------------------------------------------------------------ host: build plan


def _deg_slabs(entries):
    """Assign signatures to slabs so each slab's source-pixel union <= 128."""
    slabs = []
    rows, sigs = {}, []
    for bloc, si, tab in entries:
        new = [(bloc, rr) for rr, _ in tab if (bloc, rr) not in rows]
        if len(rows) + len(new) > 128:
            slabs.append((rows, sigs))
            rows, sigs = {}, []
            new = [(bloc, rr) for rr, _ in tab]
        for key in new:
            rows[key] = len(rows)
        sigs.append((bloc, si))
    slabs.append((rows, sigs))
    return slabs


def _dense_sources(geo, r0, r1):
    """Sorted unique corner source rows for dest rows [r0, r1) (clipped)."""
    x0i, y0i = geo[0], geo[1]
    s = set()
    for r in range(max(r0, 0), min(r1, H)):
        for cx in range(W):
            p = r * W + cx
            for sm in range(SN):
                y0, x0 = int(y0i[p, sm]), int(x0i[p, sm])
                x1, y1 = min(x0 + 1, W - 1), min(y0 + 1, H - 1)
                s.add(y0 * W + x0); s.add(y0 * W + x1)
                s.add(y1 * W + x0); s.add(y1 * W + x1)
    return sorted(s)


def make_plan(x, source_intrinsics, target_intrinsics, source_pose,
              target_pose, conv_w, conv_b):
    Fs = fundamental_np(np.asarray(source_intrinsics, np.float32),
                        np.asarray(target_intrinsics, np.float32),
                        np.asarray(source_pose, np.float32),
                        np.asarray(target_pose, np.float32))
    x = np.asarray(x, np.float32)
    xT16 = [np.ascontiguousarray(x[b].reshape(C, HW).T).astype(np.float16)
            for b in range(B)]

    degs, denses = [], []
    pi_used = 0
    for b in range(B):
        geo = geometry(Fs[b])
        res = classify(geo)
        if res is not None:
            siginv, tables = res
            pinv, ptab = patterns(siginv)
            npat = ptab.shape[0]
            if npat <= NPAT_MAX and pi_used + npat <= PI_MAX:
                degs.append(dict(gb=b, siginv=siginv, tables=tables,
                                 pinv=pinv, ptab=ptab, pi_off=pi_used))
                pi_used += npat
                continue
        denses.append(dict(gb=b, geo=geo))
    pi_tot = pi_used

    # ---- degenerate global data (same for all cores) ----
    RGROUP_CAP = 512
    rgroups = []
    cur, cur_n = [], 0
    for bi, d in enumerate(degs):
        n9 = 9 * d['ptab'].shape[0]
        if cur and cur_n + n9 > RGROUP_CAP:
            rgroups.append(dict(bis=cur, ncols=cur_n))
            cur, cur_n = [], 0
        cur.append(bi)
        cur_n += n9
    if cur:
        rgroups.append(dict(bis=cur, ncols=cur_n))

    sg_blocks, sr_blocks = [], []
    for g in rgroups:
        entries = []
        for bloc, bi in enumerate(g['bis']):
            for si, tab in enumerate(degs[bi]['tables']):
                entries.append((bloc, si, tab))
        slabs = _deg_slabs(entries)
        g['nslab'] = len(slabs)
        g['q0'] = degs[g['bis'][0]]['pi_off']
        g['nq'] = sum(degs[bi]['ptab'].shape[0] for bi in g['bis'])
        nq = g['nq']
        for rows, sigs in slabs:
            slab = np.zeros((128, C), dtype=np.float16)
            for (bloc, rr), idx in rows.items():
                slab[idx] = xT16[degs[g['bis'][bloc]]['gb']][rr]
            SR = np.zeros((128, g['ncols']), dtype=np.float32)
            sigset = set(sigs)
            for bloc, bi in enumerate(g['bis']):
                d = degs[bi]
                ptab = d['ptab']
                qb = d['pi_off'] - g['q0']
                for pi in range(ptab.shape[0]):
                    for tap in range(9):
                        si = ptab[pi, tap]
                        if si >= 0 and (bloc, si) in sigset:
                            for rr, ww in d['tables'][si]:
                                SR[rows[(bloc, rr)],
                                   tap * nq + qb + pi] += ww
            sg_blocks.append(slab)
            sr_blocks.append(SR.astype(np.float16))
    sg = (np.concatenate(sg_blocks, axis=1) if sg_blocks
          else np.zeros((128, 0), np.float16))
    sr = (np.concatenate(sr_blocks, axis=1) if sr_blocks
          else np.zeros((128, 0), np.float16))

    # ---- dense per-core per-window slabs & sampling matrices ----
    core_src = []      # [core][di][win] -> sorted source list
    nslw = [1] * len(WINS)
    for r in range(NCORE):
        per_d = []
        for d in denses:
            per_w = []
            for wi, (w0, wn) in enumerate(WINS):
                sl = _dense_sources(d['geo'], 8 * r + w0, 8 * r + w0 + wn)
                per_w.append(sl)
                nslw[wi] = max(nslw[wi], (len(sl) + 127) // 128)
            per_d.append(per_w)
        core_src.append(per_d)
    NSLW = tuple(nslw)

    # ---- weights ----
    Wl = np.zeros((128, 3 * 9 * C), dtype=np.float16)
    cw = np.asarray(conv_w, np.float32)
    cb = np.asarray(conv_b, np.float32)
    for kc, (koff, ksz) in enumerate(MB):
        for tap in range(9):
            dy, dx = tap // 3, tap % 3
            Wl[0:ksz, kc * 9 * C + tap * C: kc * 9 * C + tap * C + C] = \
                cw[:, koff:koff + ksz, dy, dx].T.astype(np.float16)
    # paired kc=2 weights for the dense conv: partitions 0:64 = dy=0 tap,
    # 64:128 = dy=1 tap (read through the +1-row shifted duplicate plane)
    Wl2 = np.zeros((128, 3 * C), dtype=np.float16)
    for dxi in range(3):
        Wl2[0:64, dxi * C: dxi * C + C] = \
            cw[:, 256:320, 0, dxi].T.astype(np.float16)
        Wl2[64:128, dxi * C: dxi * C + C] = \
            cw[:, 256:320, 1, dxi].T.astype(np.float16)
    # paired kc=2 weights for the T matmul: tap pair (2s, 2s+1) stacked on
    # the partition dim (matches rsb2's paired R layout)
    Wl2t = np.zeros((128, 5 * C), dtype=np.float16)
    for s in range(5):
        t0 = 2 * s
        Wl2t[0:64, s * C:(s + 1) * C] = \
            cw[:, 256:320, t0 // 3, t0 % 3].T.astype(np.float16)
        if t0 + 1 < 9:
            Wl2t[64:128, s * C:(s + 1) * C] = \
                cw[:, 256:320, (t0 + 1) // 3, (t0 + 1) % 3].T.astype(np.float16)
    # paired kc=2 dy=2 weights: partitions 0:64 = dx=0 tap, 64:128 = dx=1
    # (read through the +1-column shifted duplicate plane)
    Wl2x = np.zeros((128, C), dtype=np.float16)
    Wl2x[0:64, :] = cw[:, 256:320, 2, 0].T.astype(np.float16)
    Wl2x[64:128, :] = cw[:, 256:320, 2, 1].T.astype(np.float16)
    bias = np.zeros((128, 3), dtype=np.float32)
    for mc, (moff, msz) in enumerate(MB):
        bias[0:msz, mc] = cb[moff:moff + msz]

    # ---- per-core in_maps ----
    ndeg, ndense = len(degs), len(denses)
    slots = [d['gb'] for d in degs] + [d['gb'] for d in denses]
    in_maps = []
    for r in range(NCORE):
        m = {"wl": Wl, "wl2": Wl2, "wl2t": Wl2t, "wl2x": Wl2x,
             "bias": bias}
        if ndeg:
            m["sg"] = sg
            m["sr"] = sr
            e = np.zeros((128, ndeg * SW), dtype=np.float16)
            for bi, d in enumerate(degs):
                pidx = d['pinv'].reshape(H, W)[8 * r: 8 * r + STRIP].ravel()
                e[d['pi_off'] + pidx,
                  bi * SW + np.arange(SW)] = 1.0
            m["e_mat"] = e
            # mc2 batch-pair overlay: pair (2p, 2p+1) columns summed
            # (disjoint pi rows make the overlay separable by masking)
            npair = ndeg // 2
            e2 = np.zeros((128, (npair + ndeg % 2) * SW), dtype=np.float16)
            for p in range(npair):
                e2[:, p * SW:(p + 1) * SW] = (
                    e[:, 2 * p * SW:(2 * p + 1) * SW]
                    + e[:, (2 * p + 1) * SW:(2 * p + 2) * SW])
            if ndeg % 2:
                e2[:, npair * SW:] = e[:, (ndeg - 1) * SW:ndeg * SW]
            m["e2"] = e2
            pioff = [0]
            for d in degs:
                pioff.append(pioff[-1] + d['ptab'].shape[0])
            dd = np.zeros((128, max(npair, 1) * 144), dtype=np.float16)
            for p in range(npair):
                for half in range(2):
                    bi = 2 * p + half
                    for q in range(pioff[bi], pioff[bi + 1]):
                        dd[q, p * 144 + half * 72 + q] = 1.0
            m["dd"] = dd
        if ndense:
            sds, sss = [], []
            for di, d in enumerate(denses):
                for wi, (w0, wn) in enumerate(WINS):
                    sl = core_src[r][di][wi]
                    rowmap = {rr: i for i, rr in enumerate(sl)}
                    nsl = NSLW[wi]
                    slab = np.zeros((nsl * 128, C), dtype=np.float16)
                    if sl:
                        slab[:len(sl)] = xT16[d['gb']][np.array(sl)]
                    sds.append(slab.reshape(nsl, 128, C).transpose(1, 0, 2)
                               .reshape(128, nsl * C))
                    Smat = np.zeros((nsl * 128, wn * W), dtype=np.float32)
                    for lr in range(wn):
                        row = 8 * r + w0 + lr
                        if not (0 <= row < H):
                            continue
                        for cx in range(W):
                            p = row * W + cx
                            for sm in range(SN):
                                for rr, ww in _corners(d['geo'], p, sm).items():
                                    Smat[rowmap[rr], lr * W + cx] += ww
                    sss.append(Smat.astype(np.float16).reshape(nsl, 128, wn * W)
                               .transpose(1, 0, 2).reshape(128, nsl * wn * W))
            m["sd"] = np.concatenate(sds, axis=1)
            m["ss"] = np.concatenate(sss, axis=1)
        in_maps.append(m)

    struct = (pi_tot, NSLW,
              tuple((d['gb'], d['ptab'].shape[0]) for d in degs),
              tuple((g['ncols'], g['nslab'], g['q0'], g['nq'])
                    for g in rgroups),
              tuple(d['gb'] for d in denses))
    return in_maps, struct, slots


# ------------------------------------------------------------- bass program

_NC_CACHE = {}


def build_program(reps, struct):
    key = (reps, struct)
    if key in _NC_CACHE:
        return _NC_CACHE[key]
    import concourse.bacc as bacc
    import concourse.mybir as mybir
    from concourse.tile import TileContext

    fp16 = mybir.dt.float16
    f32 = mybir.dt.float32
    pi_tot, NSLW, degs, rgroups, dense_gbs = struct
    ndeg, ndense = len(degs), len(dense_gbs)
    NB = ndeg + ndense
    NSG = sum(ns for _, ns, _, _ in rgroups)
    SRC = sum(nc_ * ns for nc_, ns, _, _ in rgroups)
    NSLT = sum(NSLW)                      # total slabs per dense batch
    SSW = sum(NSLW[w] * WINS[w][1] * W for w in range(len(WINS)))

    nc = bacc.Bacc(target_bir_lowering=False)
    wl_d = nc.dram_tensor("wl", [128, 3 * 9 * C], fp16, kind="ExternalInput")
    wl2_d = nc.dram_tensor("wl2", [128, 3 * C], fp16, kind="ExternalInput")
    wl2t_d = nc.dram_tensor("wl2t", [128, 5 * C], fp16, kind="ExternalInput")
    wl2x_d = nc.dram_tensor("wl2x", [128, C], fp16, kind="ExternalInput")
    bias_d = nc.dram_tensor("bias", [128, 3], f32, kind="ExternalInput")
    if ndeg:
        sg_d = nc.dram_tensor("sg", [128, NSG * C], fp16, kind="ExternalInput")
        sr_d = nc.dram_tensor("sr", [128, SRC], fp16, kind="ExternalInput")
        e_d = nc.dram_tensor("e_mat", [128, ndeg * SW], fp16,
                             kind="ExternalInput")
        e2_d = nc.dram_tensor(
            "e2", [128, (ndeg // 2 + ndeg % 2) * SW], fp16,
            kind="ExternalInput")
        dd_d = nc.dram_tensor("dd", [128, max(ndeg // 2, 1) * 144], fp16,
                              kind="ExternalInput")
    if ndense:
        sd_d = nc.dram_tensor("sd", [128, ndense * NSLT * C], fp16,
                              kind="ExternalInput")
        ss_d = nc.dram_tensor("ss", [128, ndense * SSW], fp16,
                              kind="ExternalInput")
    ob01_d = nc.dram_tensor("ob01", [128, NB * 2 * SW], fp16,
                            kind="ExternalOutput")
    ob2_d = nc.dram_tensor("ob2", [64, NB * SW], fp16, kind="ExternalOutput")

    with TileContext(nc) as tc:
        with tc.tile_pool(name="const", bufs=1) as constp, \
             tc.tile_pool(name="inp", bufs=2) as inp, \
             tc.tile_pool(name="sdp", bufs=2) as sdp, \
             tc.tile_pool(name="work", bufs=2) as work, \
             tc.tile_pool(name="smpp", bufs=2) as smpp, \
             tc.tile_pool(name="outp", bufs=3) as outp, \
             tc.tile_pool(name="psA", bufs=3, space="PSUM") as psA, \
             tc.tile_pool(name="psB", bufs=5, space="PSUM") as psB:
            wl = constp.tile([128, 3 * 9 * C], fp16)
            nc.sync.dma_start(out=wl[:], in_=wl_d[:])
            wl2 = constp.tile([128, 3 * C], fp16)
            nc.sync.dma_start(out=wl2[:], in_=wl2_d[:])
            wl2t = constp.tile([128, 5 * C], fp16)
            nc.sync.dma_start(out=wl2t[:], in_=wl2t_d[:])
            wl2x = constp.tile([128, C], fp16)
            nc.sync.dma_start(out=wl2x[:], in_=wl2x_d[:])
            bias_t = constp.tile([128, 3], f32)
            nc.sync.dma_start(out=bias_t[:], in_=bias_d[:])

            def body(_it):
                ob01 = outp.tile([128, NB, 2, SW], fp16, name="ob01",
                                 tag="ob01")
                ob2 = outp.tile([64, NB, SW], fp16, name="ob2", tag="ob2")

                # ---------- input DMAs ----------
                if ndeg:
                    sg = inp.tile([128, NSG * C], fp16, name="sg", tag="sg")
                    nc.sync.dma_start(out=sg[:], in_=sg_d[:])
                    sr = inp.tile([128, SRC], fp16, name="sr", tag="sr")
                    nc.sync.dma_start(out=sr[:], in_=sr_d[:])
                if ndense:
                    sd = sdp.tile([128, ndense * NSLT * C], fp16, name="sd",
                                  tag="sd")
                    nc.sync.dma_start(out=sd[:], in_=sd_d[:])
                    ss = sdp.tile([128, ndense * SSW], fp16, name="ss",
                                  tag="ss")
                    nc.sync.dma_start(out=ss[:], in_=ss_d[:])
                if ndeg:
                    # needed only by the late expansion matmuls
                    e = inp.tile([128, ndeg * SW], fp16, name="e", tag="e")
                    nc.sync.dma_start(out=e[0:pi_tot, :],
                                      in_=e_d[0:pi_tot, :])
                    e2 = inp.tile([128, (ndeg // 2 + ndeg % 2) * SW], fp16,
                                  name="e2", tag="e2")
                    nc.sync.dma_start(out=e2[0:pi_tot, :],
                                      in_=e2_d[0:pi_tot, :])
                    dd = inp.tile([128, max(ndeg // 2, 1) * 144], fp16,
                                  name="dd", tag="dd")
                    nc.sync.dma_start(out=dd[0:pi_tot, :],
                                      in_=dd_d[0:pi_tot, :])

                # ---------- degenerate path: R ----------
                if ndeg:
                    # kc0/kc1 R in [ksz, kc, tap, q]; kc2 R pairwise:
                    # rsb2[0:64, s, q] = tap 2s, rsb2[64:128, s, q] = tap 2s+1
                    rsb = work.tile([128, 2, 9, pi_tot], fp16, name="rsb",
                                    tag="rsb")
                    rsb2 = work.tile([128, 5, pi_tot], fp16, name="rsb2",
                                     tag="rsb2")
                    sgo, sro = 0, 0
                    for gi, (ncols, nsl, q0, nq) in enumerate(rgroups):
                        for kc, (koff, ksz) in enumerate(MB):
                            ps_r = psB.tile([128, 512], f32,
                                            name=f"psr{gi}_{kc}", tag="psB")
                            for j in range(nsl):
                                nc.tensor.matmul(
                                    ps_r[0:ksz, 0:ncols],
                                    sg[:, (sgo + j) * C + koff:
                                       (sgo + j) * C + koff + ksz],
                                    sr[:, sro + j * ncols:
                                       sro + (j + 1) * ncols],
                                    start=(j == 0), stop=(j == nsl - 1))
                            psq = ps_r[:, 0:ncols].rearrange(
                                "p (t q) -> p t q", t=9)
                            if kc < 2:
                                eng = nc.vector.tensor_copy if kc == 0 \
                                    else nc.scalar.copy
                                eng(rsb[0:ksz, kc, :, q0:q0 + nq], psq[0:ksz])
                            else:
                                nc.vector.tensor_copy(
                                    rsb2[0:64, :, q0:q0 + nq],
                                    psq[0:64, 0:9:2])
                                nc.scalar.copy(
                                    rsb2[64:128, 0:4, q0:q0 + nq],
                                    psq[0:64, 1:9:2])
                        sgo += nsl
                        sro += nsl * ncols

                # ---------- dense sampling: 3 windows ----------
                smps = []
                if ndense:
                    for di in range(ndense):
                        # plane row i = image row 8r-1+i; cols 1..64 = image
                        smp = smpp.tile([128, 4, 10, 66], fp16,
                                        name=f"smp{di}", tag=f"smp{di}")
                        nc.gpsimd.memset(smp[:, :, :, 0:1], 0.0)
                        nc.gpsimd.memset(smp[:, :, :, 65:66], 0.0)
                        nc.gpsimd.memset(smp[64:128, 3, 0:8, 64:65], 0.0)
                        smps.append(smp)

                    for di in range(ndense):
                        smp = smps[di]
                        sdo = di * NSLT * C
                        sso = di * SSW
                        # kc2 (64-wide out) of windows 0/1 runs as two
                        # interleaved accumulation chains in opposite PE
                        # column quadrants (~2x overlap); window 2 inline
                        k2t = [psA.tile([128, 512], f32, name=f"pk2{di}_{i}",
                                        tag="psA") for i in range(3)]
                        k2mm = [[], [], []]
                        sdo0, sso0 = sdo, sso
                        for wi, (w0, wn) in enumerate(WINS):
                            px = wn * W
                            nsl = NSLW[wi]
                            pw = psA.tile([128, 512], f32,
                                          name=f"psw{di}_{wi}", tag="psA")
                            qrow = 64 if wi == 1 else 0
                            dsts = [pw[0:128, 0:px], pw[0:128, px:2 * px],
                                    k2t[wi][qrow:qrow + 64, 0:px]]
                            for kc, (koff, ksz) in enumerate(MB):
                                for j in range(nsl):
                                    args = (
                                        dsts[kc][0:ksz, :],
                                        sd[:, sdo + j * C + koff:
                                           sdo + j * C + koff + ksz],
                                        ss[:, sso + j * px:sso + (j + 1) * px])
                                    if kc < 2:
                                        nc.tensor.matmul(
                                            *args, start=(j == 0),
                                            stop=(j == nsl - 1))
                                    else:
                                        k2mm[wi].append(
                                            (args, j == 0, j == nsl - 1))
                            r0 = w0 + 1   # plane row of window start
                            pv = pw[:, 0:2 * px].rearrange(
                                "p (k r c) -> p k r c", k=2, r=wn)
                            nc.vector.tensor_copy(
                                smp[0:128, 0:2, r0:r0 + wn, 1:65],
                                pv[0:128])
                            sdo += nsl * C
                            sso += nsl * px
                        # interleave win0/win1 kc2 chains, then win2
                        for j in range(max(len(k2mm[0]), len(k2mm[1]))):
                            for wi in (0, 1):
                                if j < len(k2mm[wi]):
                                    args, st_, sp_ = k2mm[wi][j]
                                    nc.tensor.matmul(
                                        *args, start=st_, stop=sp_,
                                        tile_position=(0, 64 if wi else 0))
                        for args, st_, sp_ in k2mm[2]:
                            nc.tensor.matmul(*args, start=st_, stop=sp_)
                        for wi, (w0, wn) in enumerate(WINS):
                            px = wn * W
                            r0 = w0 + 1
                            qrow = 64 if wi == 1 else 0
                            nc.scalar.copy(
                                smp[0:64, 2, r0:r0 + wn, 1:65],
                                k2t[wi][qrow:qrow + 64, 0:px].rearrange(
                                    "p (r c) -> p r c", r=wn))
                        # +1-row shifted duplicate of kc=2 plane in parts
                        # 64:128 (for the paired dy0/dy1 conv taps)
                        nc.gpsimd.tensor_copy(smp[64:128, 2, 0:9, 1:65],
                                              smp[0:64, 2, 1:10, 1:65])
                        # plane 3: kc2 rows 2:10; parts 64:128 shifted +1 col
                        # (for the paired dy=2 dx=0/dx=1 conv taps)
                        nc.gpsimd.tensor_copy(smp[0:64, 3, 0:8, 1:65],
                                              smp[0:64, 2, 2:10, 1:65])
                        nc.gpsimd.tensor_copy(smp[64:128, 3, 0:8, 0:65],
                                              smp[0:64, 2, 2:10, 1:66])

                # ---------- degenerate path: T ----------
                if ndeg:
                    ps_t = psB.tile([128, 512], f32, name="ps_t", tag="psB")
                    k = 0
                    for kc, (koff, ksz) in enumerate(MB[:2]):
                        for tap in range(9):
                            nc.tensor.matmul(
                                ps_t[0:pi_tot, 0:C],
                                rsb[0:ksz, kc, tap, :],
                                wl[0:ksz, kc * 9 * C + tap * C:
                                   kc * 9 * C + tap * C + C],
                                start=(k == 0), stop=False)
                            k += 1
                    for s in range(5):
                        pp = 128 if s < 4 else 64
                        nc.tensor.matmul(
                            ps_t[0:pi_tot, 0:C],
                            rsb2[0:pp, s, :],
                            wl2t[0:pp, s * C:(s + 1) * C],
                            start=False, stop=(s == 4))
                    tsb = work.tile([128, C], fp16, name="tsb", tag="tsb")
                    nc.scalar.copy(tsb[0:pi_tot, :], ps_t[0:pi_tot, 0:C])
                    # masked pair stationary for mc2: pair p cols 0:64 =
                    # tsb mc2 rows of batch 2p (others zeroed), cols 64:128 =
                    # batch 2p+1 — built with diagonal-mask matmuls since
                    # partition offsets must be 32-aligned for direct copies
                    npair = ndeg // 2
                    tsb2 = work.tile([128, max(npair, 1) * 128], fp16,
                                     name="tsb2", tag="tsb2")
                    for p in range(npair):
                        ps_m = psB.tile([128, 512], f32, name=f"psm{p}",
                                        tag="psB")
                        for half in range(2):
                            nc.tensor.matmul(
                                ps_m[0:pi_tot, half * 64:half * 64 + 64],
                                dd[0:pi_tot, p * 144 + half * 72:
                                   p * 144 + half * 72 + 72],
                                tsb[0:pi_tot, 256:320],
                                start=True, stop=True)
                        nc.vector.tensor_copy(tsb2[0:pi_tot,
                                                   p * 128:(p + 1) * 128],
                                              ps_m[0:pi_tot, 0:128])

                # ---------- expansion + dense conv, interleaved per mc ----
                for mc, (moff, msz) in enumerate(MB):
                    if ndeg and mc < 2:
                        for bi in range(ndeg):
                            ps_e = psB.tile([128, 512], f32,
                                            name=f"pse{mc}_{bi}", tag="psB")
                            nc.tensor.matmul(
                                ps_e[0:msz, :],
                                tsb[0:pi_tot, moff:moff + msz],
                                e[0:pi_tot, bi * SW:(bi + 1) * SW],
                                start=True, stop=True)
                            dst = ob01[0:msz, bi, mc, :]
                            if bi % 2 == 0:
                                nc.vector.tensor_scalar(
                                    dst, ps_e[0:msz, :],
                                    bias_t[0:msz, mc:mc + 1], 0.0,
                                    mybir.AluOpType.add,
                                    mybir.AluOpType.max)
                            else:
                                nc.scalar.activation(
                                    dst, ps_e[0:msz, :],
                                    mybir.ActivationFunctionType.Relu,
                                    bias=bias_t[0:msz, mc:mc + 1])
                    elif ndeg:
                        for p in range(npair):
                            ps_e = psB.tile([128, 512], f32,
                                            name=f"pse2_{p}", tag="psB")
                            nc.tensor.matmul(
                                ps_e[0:128, :],
                                tsb2[0:pi_tot, p * 128:(p + 1) * 128],
                                e2[0:pi_tot, p * SW:(p + 1) * SW],
                                start=True, stop=True)
                            nc.vector.tensor_scalar(
                                ob2[0:64, 2 * p, :], ps_e[0:64, :],
                                bias_t[0:64, 2:3], 0.0,
                                mybir.AluOpType.add, mybir.AluOpType.max)
                            nc.scalar.activation(
                                ob2[0:64, 2 * p + 1, :], ps_e[64:128, :],
                                mybir.ActivationFunctionType.Relu,
                                bias=bias_t[0:64, 2:3])
                        if ndeg % 2:
                            bi = ndeg - 1
                            ps_e = psB.tile([128, 512], f32,
                                            name="pse2_last", tag="psB")
                            nc.tensor.matmul(
                                ps_e[0:msz, :],
                                tsb[0:pi_tot, moff:moff + msz],
                                e[0:pi_tot, bi * SW:(bi + 1) * SW],
                                start=True, stop=True)
                            nc.vector.tensor_scalar(
                                ob2[0:msz, bi, :], ps_e[0:msz, :],
                                bias_t[0:msz, 2:3], 0.0,
                                mybir.AluOpType.add, mybir.AluOpType.max)
                    for di in range(ndense):
                        smp = smps[di]
                        ps_c = psB.tile([128, 512], f32,
                                        name=f"psc{di}_{mc}", tag="psB")
                        # build the 23 contraction chunks as (stat, mov)
                        chunks = []
                        for kc, (koff, ksz) in enumerate(MB[:2]):
                            for tap in range(9):
                                dy, dx = tap // 3, tap % 3
                                chunks.append((
                                    wl[0:ksz,
                                       kc * 9 * C + tap * C + moff:
                                       kc * 9 * C + tap * C + moff + msz],
                                    smp[0:ksz, kc, dy:dy + 8, dx:dx + 64]))
                        for dx in range(3):
                            chunks.append((
                                wl2[0:128, dx * C + moff:
                                    dx * C + moff + msz],
                                smp[0:128, 2, 0:8, dx:dx + 64]))
                        chunks.append((wl2x[0:128, moff:moff + msz],
                                       smp[0:128, 3, 0:8, 0:64]))
                        chunks.append((
                            wl[0:64, 2 * 9 * C + 8 * C + moff:
                               2 * 9 * C + 8 * C + moff + msz],
                            smp[0:64, 2, 2:10, 2:66]))
                        if mc < 2:
                            # full-width output: single accumulation chain
                            for k, (st, mv) in enumerate(chunks):
                                nc.tensor.matmul(
                                    ps_c[0:msz, :], st, mv,
                                    start=(k == 0), stop=(k == 22))
                            dst = ob01[0:msz, ndeg + di, mc, :]
                            if mc == 1:
                                nc.vector.tensor_scalar(
                                    dst, ps_c[0:msz, :],
                                    bias_t[0:msz, mc:mc + 1], 0.0,
                                    mybir.AluOpType.add, mybir.AluOpType.max)
                            else:
                                nc.scalar.activation(
                                    dst, ps_c[0:msz, :],
                                    mybir.ActivationFunctionType.Relu,
                                    bias=bias_t[0:msz, mc:mc + 1])
                        else:
                            # 64-wide output: two half-chains in opposite PE
                            # column quadrants overlap ~2x; partials summed
                            # at drain
                            ps_cb = psB.tile([128, 512], f32,
                                             name=f"pscb{di}", tag="psB")
                            ha, hb = chunks[0:12], chunks[12:23]
                            for k in range(12):
                                st, mv = ha[k]
                                nc.tensor.matmul(
                                    ps_c[0:64, :], st, mv,
                                    start=(k == 0), stop=(k == 11),
                                    tile_position=(0, 0))
                                if k < len(hb):
                                    st, mv = hb[k]
                                    nc.tensor.matmul(
                                        ps_cb[64:128, :], st, mv,
                                        start=(k == 0),
                                        stop=(k == len(hb) - 1),
                                        tile_position=(0, 64))
                            dst = ob2[0:msz, ndeg + di, :]
                            cv2 = work.tile([64, 512], fp16,
                                            name=f"cv2{di}", tag=f"cv2{di}")
                            nc.scalar.copy(cv2[:, :], ps_cb[64:128, :])
                            nc.vector.scalar_tensor_tensor(
                                dst, ps_c[0:64, :],
                                bias_t[0:64, 2:3], cv2[:, :],
                                mybir.AluOpType.add, mybir.AluOpType.add)
                            nc.gpsimd.tensor_scalar_max(dst, dst, 0.0)
                # ---------- output DMAs ----------
                # deg slices complete after the mc2 expansion drains and
                # ship while the last conv block still runs
                nc.sync.dma_start(out=ob01_d[:, 0:ndeg * 2 * SW],
                                  in_=ob01[:, 0:ndeg, :, :])
                nc.sync.dma_start(out=ob2_d[:, 0:ndeg * SW],
                                  in_=ob2[:, 0:ndeg, :])
                nc.sync.dma_start(out=ob01_d[:, ndeg * 2 * SW:],
                                  in_=ob01[:, ndeg:, :, :])
                nc.sync.dma_start(out=ob2_d[:, ndeg * SW:],
                                  in_=ob2[:, ndeg:, :])

            if reps < 0:
                # unrolled, no hardware loop (TimelineSim profiling)
                for u in range(-reps):
                    body(u)
            elif reps == 1:
                body(0)
            else:
                U = 8
                n_loop = reps // U
                hints = (mybir.EngineType.PE, mybir.EngineType.Activation,
                         mybir.EngineType.Pool, mybir.EngineType.SP,
                         mybir.EngineType.DVE)
                with tc.For_i(0, n_loop, 1, hint_engines=hints) as it:
                    for u in range(U):
                        body(u)
                for u in range(reps - n_loop * U):
                    body(u)

    nc.finalize()
    _NC_CACHE[key] = nc
    return nc


# ---------------------------------------------------------------- interface


def make_in_maps(x, source_intrinsics, target_intrinsics, source_pose,
                 target_pose, conv_w, conv_b):
    return make_plan(x, source_intrinsics, target_intrinsics, source_pose,
                     target_pose, conv_w, conv_b)


def assemble(results, slots):
    """results: per-core {"ob01": [128, NB*2*SW], "ob2": [64, NB*SW]}."""
    out = np.zeros((B, C, H, W), dtype=np.float32)
    NBl = len(slots)
    for r in range(NCORE):
        o01 = np.asarray(results[r]["ob01"]).reshape(128, NBl, 2, STRIP, W)
        o2 = np.asarray(results[r]["ob2"]).reshape(64, NBl, STRIP, W)
        for si, gb in enumerate(slots):
            out[gb, 0:128, 8 * r: 8 * r + STRIP, :] = \
                o01[:, si, 0].astype(np.float32)
            out[gb, 128:256, 8 * r: 8 * r + STRIP, :] = \
                o01[:, si, 1].astype(np.float32)
            out[gb, 256:320, 8 * r: 8 * r + STRIP, :] = \
                o2[:, si].astype(np.float32)
    return out


def kernel(x, source_intrinsics, target_intrinsics, source_pose,
           target_pose, conv_w, conv_b, _reps=1):
    from concourse.bass_utils import run_bass_kernel_spmd
    in_maps, struct, slots = make_in_maps(
        x, source_intrinsics, target_intrinsics, source_pose,
        target_pose, conv_w, conv_b)
    nc = build_program(_reps, struct)
    res = run_bass_kernel_spmd(nc, in_maps, list(range(NCORE)))
    return assemble(res.results, slots)


# revision 37
# speedup vs baseline: 1.0343x; 1.0343x over previous
"""Trainium2 Bass kernel for nn_EpipolarWarpOperator (B=8, C=320, H=W=64, S=3).

Sharding: spatial — every core computes an 8-row horizontal strip of all 8
batch outputs (the 3x3 conv is strip-separable; epipolar sampling per strip
with a 1-row halo).

Host analysis classifies each batch by its epipolar sampling map:

* pattern batches: the per-pixel bilinear sampling signature map has few
  distinct values, so the sampled image is piecewise constant and each output
  column is one of npat distinct "pattern" columns. The device computes
  R[k, tap, pi] from a host-gathered slab via tiny matmuls (kc=2 taps stored
  pairwise over the partition dim), T^T[pi, ch] = sum R_tap^T W_tap (23
  matmuls), then expands out = T^T.T @ E with a per-core 0/1 selection
  matrix E, bias+relu fused into the PSUM drains.

* dense batches: the whole image touches only a few hundred distinct bilinear
  corner pixels; the host gathers them into per-window slabs [128, C] plus
  window sampling matrices (3 windows: 4+4+2 rows covering the strip plus a
  1-row conv halo); the device runs swapped-operand matmuls accumulating the
  slabs into PSUM, then a 24-matmul-per-mc 3x3 conv over the 10-row sampled
  plane (kc=2 taps paired through a +1-row shifted duplicate) with bias+relu
  on the drain.
"""

import numpy as np

B, C, H, W = 8, 320, 64, 64
SN = 3
HW = H * W
STRIP = 8             # output rows per core
NCORE = 8
MB = [(0, 128), (128, 128), (256, 64)]   # output/input channel chunking
PI_MAX = 126          # total pattern budget
NPAT_MAX = 56         # per-batch pattern cap
NSIG_MAX = 64
SW = STRIP * W        # 512 px per strip
WINS = ((-1, 3), (2, 3), (5, 4))   # dense sampling windows (row0, nrows)

# ---------------------------------------------------------------- host: geometry


def _rodrigues_np(rv):
    theta = np.sqrt((rv * rv).sum())
    r = rv / max(theta, 1e-12)
    I = np.eye(3, dtype=np.float32)
    K = np.array([[0, -r[2], r[1]], [r[2], 0, -r[0]], [-r[1], r[0], 0]],
                 dtype=np.float32)
    R = np.cos(theta) * I + (1 - np.cos(theta)) * np.outer(r, r) + np.sin(theta) * K
    return I if theta < 1e-6 else R


def fundamental_np(Ks, Kt, ps, pt):
    Fs = []
    for b in range(Ks.shape[0]):
        Rs = _rodrigues_np(ps[b, :3].astype(np.float32))
        Rt = _rodrigues_np(pt[b, :3].astype(np.float32))
        ts_, tt_ = ps[b, 3:].astype(np.float32), pt[b, 3:].astype(np.float32)
        R_rel = Rs @ Rt.T
        t_rel = ts_ - R_rel @ tt_
        z = np.float32(0)
        skew = np.array([[z, -t_rel[2], t_rel[1]],
                         [t_rel[2], z, -t_rel[0]],
                         [-t_rel[1], t_rel[0], z]], dtype=np.float32)
        E = skew @ R_rel
        inv_Ks = np.linalg.inv(Ks[b].astype(np.float32))
        inv_Kt = np.linalg.inv(Kt[b].astype(np.float32))
        Fs.append(inv_Kt.T @ E @ inv_Ks)
    return np.stack(Fs).astype(np.float32)


def geometry(F):
    k = np.arange(HW)
    px = (k % W).astype(np.float32)
    py = (k // W).astype(np.float32)
    P = np.stack([px, py, np.ones_like(px)])
    lines = F.T.astype(np.float32) @ P
    a, b_, c = lines[0], lines[1], lines[2]
    W1, H1 = np.float32(W - 1), np.float32(H - 1)
    EPS = np.float32(1e-10)
    x1 = np.clip(-c / (a + EPS), 0.0, W1)
    x2 = np.clip(-(b_ * H1 + c) / (a + EPS), 0.0, W1)
    y1 = np.clip(-c / (b_ + EPS), 0.0, H1)
    y2 = np.clip(-(a * W1 + c) / (b_ + EPS), 0.0, H1)
    t = np.linspace(0.0, 1.0, SN, dtype=np.float32)
    sx = x1[:, None] * (1 - t) + x2[:, None] * t
    sy = y1[:, None] * (1 - t) + y2[:, None] * t
    x0 = np.floor(sx)
    y0 = np.floor(sy)
    wx = (sx - x0).astype(np.float32)
    wy = (sy - y0).astype(np.float32)
    x0i = np.clip(x0, 0, W - 1).astype(np.int32)
    y0i = np.clip(y0, 0, H - 1).astype(np.int32)
    return x0i, y0i, wx, wy


def _corners(geo, p, s):
    """4 bilinear (source pixel row, weight/3) pairs for pixel p, sample s."""
    x0i, y0i, wx, wy = geo
    y0 = int(y0i[p, s]); x0 = int(x0i[p, s])
    x1 = min(x0 + 1, W - 1); y1 = min(y0 + 1, H - 1)
    wxx = np.float32(wx[p, s]); wyy = np.float32(wy[p, s])
    third = np.float32(1.0 / 3.0)
    out = {}
    for ry, rx, ww in ((y0, x0, (1 - wxx) * (1 - wyy)),
                       (y0, x1, wxx * (1 - wyy)),
                       (y1, x0, (1 - wxx) * wyy),
                       (y1, x1, wxx * wyy)):
        rr = ry * W + rx
        out[rr] = out.get(rr, np.float32(0)) + ww * third
    return out


# ------------------------------------------------------------- host: classify


def classify(geo):
    x0i, y0i, wx, wy = geo
    key = np.concatenate([
        y0i.astype(np.float32), x0i.astype(np.float32), wx, wy], axis=1)
    kview = np.ascontiguousarray(key).view([('', key.dtype)] * key.shape[1]).ravel()
    uniq, first, inv = np.unique(kview, return_index=True, return_inverse=True)
    if len(uniq) > NSIG_MAX:
        return None
    tables = []
    for si in range(len(uniq)):
        p = int(first[si])
        contrib = {}
        for s in range(SN):
            for rr, ww in _corners(geo, p, s).items():
                contrib[rr] = contrib.get(rr, np.float32(0)) + ww
        tables.append(sorted(contrib.items()))
    return inv.astype(np.int32), tables


def patterns(siginv):
    """3x3 neighborhood patterns of the signature map (border = -1)."""
    simg = siginv.reshape(H, W)
    pad = np.pad(simg, 1, constant_values=-1)
    neigh = np.stack([pad[dy:dy + H, dx:dx + W].ravel()
                      for dy in range(3) for dx in range(3)], axis=1)
    uniq, pinv = np.unique(neigh, axis=0, return_inverse=True)
    return pinv.astype(np.int32), uniq


# ------------------------------------------------------------ host: build plan


def _deg_slabs(entries):
    """Assign signatures to slabs so each slab's source-pixel union <= 128."""
    slabs = []
    rows, sigs = {}, []
    for bloc, si, tab in entries:
        new = [(bloc, rr) for rr, _ in tab if (bloc, rr) not in rows]
        if len(rows) + len(new) > 128:
            slabs.append((rows, sigs))
            rows, sigs = {}, []
            new = [(bloc, rr) for rr, _ in tab]
        for key in new:
            rows[key] = len(rows)
        sigs.append((bloc, si))
    slabs.append((rows, sigs))
    return slabs


def _dense_sources(geo, r0, r1):
    """Sorted unique corner source rows for dest rows [r0, r1) (clipped)."""
    x0i, y0i = geo[0], geo[1]
    s = set()
    for r in range(max(r0, 0), min(r1, H)):
        for cx in range(W):
            p = r * W + cx
            for sm in range(SN):
                y0, x0 = int(y0i[p, sm]), int(x0i[p, sm])
                x1, y1 = min(x0 + 1, W - 1), min(y0 + 1, H - 1)
                s.add(y0 * W + x0); s.add(y0 * W + x1)
                s.add(y1 * W + x0); s.add(y1 * W + x1)
    return sorted(s)


def make_plan(x, source_intrinsics, target_intrinsics, source_pose,
              target_pose, conv_w, conv_b):
    Fs = fundamental_np(np.asarray(source_intrinsics, np.float32),
                        np.asarray(target_intrinsics, np.float32),
                        np.asarray(source_pose, np.float32),
                        np.asarray(target_pose, np.float32))
    x = np.asarray(x, np.float32)
    xT16 = [np.ascontiguousarray(x[b].reshape(C, HW).T).astype(np.float16)
            for b in range(B)]

    degs, denses = [], []
    pi_used = 0
    for b in range(B):
        geo = geometry(Fs[b])
        res = classify(geo)
        if res is not None:
            siginv, tables = res
            pinv, ptab = patterns(siginv)
            npat = ptab.shape[0]
            if npat <= NPAT_MAX and pi_used + npat <= PI_MAX:
                degs.append(dict(gb=b, siginv=siginv, tables=tables,
                                 pinv=pinv, ptab=ptab, pi_off=pi_used))
                pi_used += npat
                continue
        denses.append(dict(gb=b, geo=geo))
    pi_tot = pi_used

    # ---- degenerate global data (same for all cores) ----
    RGROUP_CAP = 512
    rgroups = []
    cur, cur_n = [], 0
    for bi, d in enumerate(degs):
        n9 = 9 * d['ptab'].shape[0]
        if cur and cur_n + n9 > RGROUP_CAP:
            rgroups.append(dict(bis=cur, ncols=cur_n))
            cur, cur_n = [], 0
        cur.append(bi)
        cur_n += n9
    if cur:
        rgroups.append(dict(bis=cur, ncols=cur_n))

    sg_blocks, sr_blocks = [], []
    for g in rgroups:
        entries = []
        for bloc, bi in enumerate(g['bis']):
            for si, tab in enumerate(degs[bi]['tables']):
                entries.append((bloc, si, tab))
        slabs = _deg_slabs(entries)
        g['nslab'] = len(slabs)
        g['q0'] = degs[g['bis'][0]]['pi_off']
        g['nq'] = sum(degs[bi]['ptab'].shape[0] for bi in g['bis'])
        nq = g['nq']
        for rows, sigs in slabs:
            slab = np.zeros((128, C), dtype=np.float16)
            for (bloc, rr), idx in rows.items():
                slab[idx] = xT16[degs[g['bis'][bloc]]['gb']][rr]
            SR = np.zeros((128, g['ncols']), dtype=np.float32)
            sigset = set(sigs)
            for bloc, bi in enumerate(g['bis']):
                d = degs[bi]
                ptab = d['ptab']
                qb = d['pi_off'] - g['q0']
                for pi in range(ptab.shape[0]):
                    for tap in range(9):
                        si = ptab[pi, tap]
                        if si >= 0 and (bloc, si) in sigset:
                            for rr, ww in d['tables'][si]:
                                SR[rows[(bloc, rr)],
                                   tap * nq + qb + pi] += ww
            sg_blocks.append(slab)
            sr_blocks.append(SR.astype(np.float16))
    sg = (np.concatenate(sg_blocks, axis=1) if sg_blocks
          else np.zeros((128, 0), np.float16))
    sr = (np.concatenate(sr_blocks, axis=1) if sr_blocks
          else np.zeros((128, 0), np.float16))

    # ---- dense per-core per-window slabs & sampling matrices ----
    core_src = []      # [core][di][win] -> sorted source list
    nslw = [1] * len(WINS)
    for r in range(NCORE):
        per_d = []
        for d in denses:
            per_w = []
            for wi, (w0, wn) in enumerate(WINS):
                sl = _dense_sources(d['geo'], 8 * r + w0, 8 * r + w0 + wn)
                per_w.append(sl)
                nslw[wi] = max(nslw[wi], (len(sl) + 127) // 128)
            per_d.append(per_w)
        core_src.append(per_d)
    NSLW = tuple(nslw)

    # ---- weights ----
    Wl = np.zeros((128, 3 * 9 * C), dtype=np.float16)
    cw = np.asarray(conv_w, np.float32)
    cb = np.asarray(conv_b, np.float32)
    for kc, (koff, ksz) in enumerate(MB):
        for tap in range(9):
            dy, dx = tap // 3, tap % 3
            Wl[0:ksz, kc * 9 * C + tap * C: kc * 9 * C + tap * C + C] = \
                cw[:, koff:koff + ksz, dy, dx].T.astype(np.float16)
    # paired kc=2 weights for the dense conv: partitions 0:64 = dy=0 tap,
    # 64:128 = dy=1 tap (read through the +1-row shifted duplicate plane)
    Wl2 = np.zeros((128, 3 * C), dtype=np.float16)
    for dxi in range(3):
        Wl2[0:64, dxi * C: dxi * C + C] = \
            cw[:, 256:320, 0, dxi].T.astype(np.float16)
        Wl2[64:128, dxi * C: dxi * C + C] = \
            cw[:, 256:320, 1, dxi].T.astype(np.float16)
    # paired kc=2 weights for the T matmul: tap pair (2s, 2s+1) stacked on
    # the partition dim (matches rsb2's paired R layout)
    Wl2t = np.zeros((128, 5 * C), dtype=np.float16)
    for s in range(5):
        t0 = 2 * s
        Wl2t[0:64, s * C:(s + 1) * C] = \
            cw[:, 256:320, t0 // 3, t0 % 3].T.astype(np.float16)
        if t0 + 1 < 9:
            Wl2t[64:128, s * C:(s + 1) * C] = \
                cw[:, 256:320, (t0 + 1) // 3, (t0 + 1) % 3].T.astype(np.float16)
    # paired kc=2 dy=2 weights: partitions 0:64 = dx=0 tap, 64:128 = dx=1
    # (read through the +1-column shifted duplicate plane)
    Wl2x = np.zeros((128, C), dtype=np.float16)
    Wl2x[0:64, :] = cw[:, 256:320, 2, 0].T.astype(np.float16)
    Wl2x[64:128, :] = cw[:, 256:320, 2, 1].T.astype(np.float16)
    bias = np.zeros((128, 3), dtype=np.float32)
    for mc, (moff, msz) in enumerate(MB):
        bias[0:msz, mc] = cb[moff:moff + msz]

    # ---- per-core in_maps ----
    ndeg, ndense = len(degs), len(denses)
    slots = [d['gb'] for d in degs] + [d['gb'] for d in denses]
    in_maps = []
    for r in range(NCORE):
        m = {"wl": Wl, "wl2": Wl2, "wl2t": Wl2t, "wl2x": Wl2x,
             "bias": bias}
        if ndeg:
            m["sg"] = sg
            m["sr"] = sr
            e = np.zeros((128, ndeg * SW), dtype=np.float16)
            for bi, d in enumerate(degs):
                pidx = d['pinv'].reshape(H, W)[8 * r: 8 * r + STRIP].ravel()
                e[d['pi_off'] + pidx,
                  bi * SW + np.arange(SW)] = 1.0
            m["e_mat"] = e
            # mc2 batch-pair overlay: pair (2p, 2p+1) columns summed
            # (disjoint pi rows make the overlay separable by masking)
            npair = ndeg // 2
            e2 = np.zeros((128, (npair + ndeg % 2) * SW), dtype=np.float16)
            for p in range(npair):
                e2[:, p * SW:(p + 1) * SW] = (
                    e[:, 2 * p * SW:(2 * p + 1) * SW]
                    + e[:, (2 * p + 1) * SW:(2 * p + 2) * SW])
            if ndeg % 2:
                e2[:, npair * SW:] = e[:, (ndeg - 1) * SW:ndeg * SW]
            m["e2"] = e2
            pioff = [0]
            for d in degs:
                pioff.append(pioff[-1] + d['ptab'].shape[0])
            dd = np.zeros((128, max(npair, 1) * 144), dtype=np.float16)
            for p in range(npair):
                for half in range(2):
                    bi = 2 * p + half
                    for q in range(pioff[bi], pioff[bi + 1]):
                        dd[q, p * 144 + half * 72 + q] = 1.0
            m["dd"] = dd
        if ndense:
            sds, sss = [], []
            for di, d in enumerate(denses):
                for wi, (w0, wn) in enumerate(WINS):
                    sl = core_src[r][di][wi]
                    rowmap = {rr: i for i, rr in enumerate(sl)}
                    nsl = NSLW[wi]
                    slab = np.zeros((nsl * 128, C), dtype=np.float16)
                    if sl:
                        slab[:len(sl)] = xT16[d['gb']][np.array(sl)]
                    sds.append(slab.reshape(nsl, 128, C).transpose(1, 0, 2)
                               .reshape(128, nsl * C))
                    Smat = np.zeros((nsl * 128, wn * W), dtype=np.float32)
                    for lr in range(wn):
                        row = 8 * r + w0 + lr
                        if not (0 <= row < H):
                            continue
                        for cx in range(W):
                            p = row * W + cx
                            for sm in range(SN):
                                for rr, ww in _corners(d['geo'], p, sm).items():
                                    Smat[rowmap[rr], lr * W + cx] += ww
                    sss.append(Smat.astype(np.float16).reshape(nsl, 128, wn * W)
                               .transpose(1, 0, 2).reshape(128, nsl * wn * W))
            m["sd"] = np.concatenate(sds, axis=1)
            m["ss"] = np.concatenate(sss, axis=1)
        in_maps.append(m)

    struct = (pi_tot, NSLW,
              tuple((d['gb'], d['ptab'].shape[0]) for d in degs),
              tuple((g['ncols'], g['nslab'], g['q0'], g['nq'])
                    for g in rgroups),
              tuple(d['gb'] for d in denses))
    return in_maps, struct, slots


# ------------------------------------------------------------- bass program

_NC_CACHE = {}


def build_program(reps, struct):
    key = (reps, struct)
    if key in _NC_CACHE:
        return _NC_CACHE[key]
    import concourse.bacc as bacc
    import concourse.mybir as mybir
    from concourse.tile import TileContext

    fp16 = mybir.dt.float16
    f32 = mybir.dt.float32
    pi_tot, NSLW, degs, rgroups, dense_gbs = struct
    ndeg, ndense = len(degs), len(dense_gbs)
    NB = ndeg + ndense
    NSG = sum(ns for _, ns, _, _ in rgroups)
    SRC = sum(nc_ * ns for nc_, ns, _, _ in rgroups)
    NSLT = sum(NSLW)                      # total slabs per dense batch
    SSW = sum(NSLW[w] * WINS[w][1] * W for w in range(len(WINS)))

    nc = bacc.Bacc(target_bir_lowering=False)
    wl_d = nc.dram_tensor("wl", [128, 3 * 9 * C], fp16, kind="ExternalInput")
    wl2_d = nc.dram_tensor("wl2", [128, 3 * C], fp16, kind="ExternalInput")
    wl2t_d = nc.dram_tensor("wl2t", [128, 5 * C], fp16, kind="ExternalInput")
    wl2x_d = nc.dram_tensor("wl2x", [128, C], fp16, kind="ExternalInput")
    bias_d = nc.dram_tensor("bias", [128, 3], f32, kind="ExternalInput")
    if ndeg:
        sg_d = nc.dram_tensor("sg", [128, NSG * C], fp16, kind="ExternalInput")
        sr_d = nc.dram_tensor("sr", [128, SRC], fp16, kind="ExternalInput")
        e_d = nc.dram_tensor("e_mat", [128, ndeg * SW], fp16,
                             kind="ExternalInput")
        e2_d = nc.dram_tensor(
            "e2", [128, (ndeg // 2 + ndeg % 2) * SW], fp16,
            kind="ExternalInput")
        dd_d = nc.dram_tensor("dd", [128, max(ndeg // 2, 1) * 144], fp16,
                              kind="ExternalInput")
    if ndense:
        sd_d = nc.dram_tensor("sd", [128, ndense * NSLT * C], fp16,
                              kind="ExternalInput")
        ss_d = nc.dram_tensor("ss", [128, ndense * SSW], fp16,
                              kind="ExternalInput")
    ob01_d = nc.dram_tensor("ob01", [128, NB * 2 * SW], fp16,
                            kind="ExternalOutput")
    ob2_d = nc.dram_tensor("ob2", [64, NB * SW], fp16, kind="ExternalOutput")

    with TileContext(nc) as tc:
        with tc.tile_pool(name="const", bufs=1) as constp, \
             tc.tile_pool(name="inp", bufs=2) as inp, \
             tc.tile_pool(name="sdp", bufs=2) as sdp, \
             tc.tile_pool(name="work", bufs=2) as work, \
             tc.tile_pool(name="smpp", bufs=2) as smpp, \
             tc.tile_pool(name="outp", bufs=3) as outp, \
             tc.tile_pool(name="psA", bufs=3, space="PSUM") as psA, \
             tc.tile_pool(name="psB", bufs=5, space="PSUM") as psB:
            wl = constp.tile([128, 3 * 9 * C], fp16)
            nc.sync.dma_start(out=wl[:], in_=wl_d[:])
            wl2 = constp.tile([128, 3 * C], fp16)
            nc.sync.dma_start(out=wl2[:], in_=wl2_d[:])
            wl2t = constp.tile([128, 5 * C], fp16)
            nc.sync.dma_start(out=wl2t[:], in_=wl2t_d[:])
            wl2x = constp.tile([128, C], fp16)
            nc.sync.dma_start(out=wl2x[:], in_=wl2x_d[:])
            bias_t = constp.tile([128, 3], f32)
            nc.sync.dma_start(out=bias_t[:], in_=bias_d[:])

            def body(_it):
                ob01 = outp.tile([128, NB, 2, SW], fp16, name="ob01",
                                 tag="ob01")
                ob2 = outp.tile([64, NB, SW], fp16, name="ob2", tag="ob2")

                # ---------- input DMAs ----------
                if ndeg:
                    sg = inp.tile([128, NSG * C], fp16, name="sg", tag="sg")
                    nc.sync.dma_start(out=sg[:], in_=sg_d[:])
                    sr = inp.tile([128, SRC], fp16, name="sr", tag="sr")
                    nc.sync.dma_start(out=sr[:], in_=sr_d[:])
                if ndense:
                    sd = sdp.tile([128, ndense * NSLT * C], fp16, name="sd",
                                  tag="sd")
                    nc.sync.dma_start(out=sd[:], in_=sd_d[:])
                    ss = sdp.tile([128, ndense * SSW], fp16, name="ss",
                                  tag="ss")
                    nc.sync.dma_start(out=ss[:], in_=ss_d[:])
                if ndeg:
                    # needed only by the late expansion matmuls
                    e = inp.tile([128, ndeg * SW], fp16, name="e", tag="e")
                    nc.sync.dma_start(out=e[0:pi_tot, :],
                                      in_=e_d[0:pi_tot, :])
                    e2 = inp.tile([128, (ndeg // 2 + ndeg % 2) * SW], fp16,
                                  name="e2", tag="e2")
                    nc.sync.dma_start(out=e2[0:pi_tot, :],
                                      in_=e2_d[0:pi_tot, :])
                    dd = inp.tile([128, max(ndeg // 2, 1) * 144], fp16,
                                  name="dd", tag="dd")
                    nc.sync.dma_start(out=dd[0:pi_tot, :],
                                      in_=dd_d[0:pi_tot, :])

                # ---------- degenerate path: R ----------
                if ndeg:
                    # kc0/kc1 R in [ksz, kc, tap, q]; kc2 R pairwise:
                    # rsb2[0:64, s, q] = tap 2s, rsb2[64:128, s, q] = tap 2s+1
                    rsb = work.tile([128, 2, 9, pi_tot], fp16, name="rsb",
                                    tag="rsb")
                    rsb2 = work.tile([128, 5, pi_tot], fp16, name="rsb2",
                                     tag="rsb2")
                    sgo, sro = 0, 0
                    for gi, (ncols, nsl, q0, nq) in enumerate(rgroups):
                        for kc, (koff, ksz) in enumerate(MB):
                            ps_r = psB.tile([128, 512], f32,
                                            name=f"psr{gi}_{kc}", tag="psB")
                            for j in range(nsl):
                                nc.tensor.matmul(
                                    ps_r[0:ksz, 0:ncols],
                                    sg[:, (sgo + j) * C + koff:
                                       (sgo + j) * C + koff + ksz],
                                    sr[:, sro + j * ncols:
                                       sro + (j + 1) * ncols],
                                    start=(j == 0), stop=(j == nsl - 1))
                            psq = ps_r[:, 0:ncols].rearrange(
                                "p (t q) -> p t q", t=9)
                            if kc < 2:
                                eng = nc.vector.tensor_copy if kc == 0 \
                                    else nc.scalar.copy
                                eng(rsb[0:ksz, kc, :, q0:q0 + nq], psq[0:ksz])
                            else:
                                nc.vector.tensor_copy(
                                    rsb2[0:64, :, q0:q0 + nq],
                                    psq[0:64, 0:9:2])
                                nc.scalar.copy(
                                    rsb2[64:128, 0:4, q0:q0 + nq],
                                    psq[0:64, 1:9:2])
                        sgo += nsl
                        sro += nsl * ncols

                # ---------- dense sampling: 3 windows ----------
                smps = []
                if ndense:
                    for di in range(ndense):
                        # plane row i = image row 8r-1+i; cols 1..64 = image
                        smp = smpp.tile([128, 4, 10, 66], fp16,
                                        name=f"smp{di}", tag=f"smp{di}")
                        nc.gpsimd.memset(smp[:, :, :, 0:1], 0.0)
                        nc.gpsimd.memset(smp[:, :, :, 65:66], 0.0)
                        nc.gpsimd.memset(smp[64:128, 3, 0:8, 64:65], 0.0)
                        smps.append(smp)

                    for di in range(ndense):
                        smp = smps[di]
                        sdo = di * NSLT * C
                        sso = di * SSW
                        # kc2 (64-wide out) of windows 0/1 runs as two
                        # interleaved accumulation chains in opposite PE
                        # column quadrants (~2x overlap); window 2 inline
                        k2t = [psA.tile([128, 512], f32, name=f"pk2{di}_{i}",
                                        tag="psA") for i in range(3)]
                        k2mm = [[], [], []]
                        sdo0, sso0 = sdo, sso
                        for wi, (w0, wn) in enumerate(WINS):
                            px = wn * W
                            nsl = NSLW[wi]
                            pw = psA.tile([128, 512], f32,
                                          name=f"psw{di}_{wi}", tag="psA")
                            qrow = 64 if wi == 1 else 0
                            dsts = [pw[0:128, 0:px], pw[0:128, px:2 * px],
                                    k2t[wi][qrow:qrow + 64, 0:px]]
                            for kc, (koff, ksz) in enumerate(MB):
                                for j in range(nsl):
                                    args = (
                                        dsts[kc][0:ksz, :],
                                        sd[:, sdo + j * C + koff:
                                           sdo + j * C + koff + ksz],
                                        ss[:, sso + j * px:sso + (j + 1) * px])
                                    if kc < 2:
                                        nc.tensor.matmul(
                                            *args, start=(j == 0),
                                            stop=(j == nsl - 1))
                                    else:
                                        k2mm[wi].append(
                                            (args, j == 0, j == nsl - 1))
                            r0 = w0 + 1   # plane row of window start
                            pv = pw[:, 0:2 * px].rearrange(
                                "p (k r c) -> p k r c", k=2, r=wn)
                            nc.vector.tensor_copy(
                                smp[0:128, 0:2, r0:r0 + wn, 1:65],
                                pv[0:128])
                            sdo += nsl * C
                            sso += nsl * px
                        # interleave win0/win1 kc2 chains, then win2
                        for j in range(max(len(k2mm[0]), len(k2mm[1]))):
                            for wi in (0, 1):
                                if j < len(k2mm[wi]):
                                    args, st_, sp_ = k2mm[wi][j]
                                    nc.tensor.matmul(
                                        *args, start=st_, stop=sp_,
                                        tile_position=(0, 64 if wi else 0))
                        for args, st_, sp_ in k2mm[2]:
                            nc.tensor.matmul(*args, start=st_, stop=sp_)
                        for wi, (w0, wn) in enumerate(WINS):
                            px = wn * W
                            r0 = w0 + 1
                            qrow = 64 if wi == 1 else 0
                            nc.scalar.copy(
                                smp[0:64, 2, r0:r0 + wn, 1:65],
                                k2t[wi][qrow:qrow + 64, 0:px].rearrange(
                                    "p (r c) -> p r c", r=wn))
                        # +1-row shifted duplicate of kc=2 plane in parts
                        # 64:128 (for the paired dy0/dy1 conv taps)
                        nc.gpsimd.tensor_copy(smp[64:128, 2, 0:9, 1:65],
                                              smp[0:64, 2, 1:10, 1:65])
                        # plane 3: kc2 rows 2:10; parts 64:128 shifted +1 col
                        # (for the paired dy=2 dx=0/dx=1 conv taps)
                        nc.gpsimd.tensor_copy(smp[0:64, 3, 0:8, 1:65],
                                              smp[0:64, 2, 2:10, 1:65])
                        nc.gpsimd.tensor_copy(smp[64:128, 3, 0:8, 0:65],
                                              smp[0:64, 2, 2:10, 1:66])

                # ---------- degenerate path: T ----------
                if ndeg:
                    ps_t = psB.tile([128, 512], f32, name="ps_t", tag="psB")
                    k = 0
                    for kc, (koff, ksz) in enumerate(MB[:2]):
                        for tap in range(9):
                            nc.tensor.matmul(
                                ps_t[0:pi_tot, 0:C],
                                rsb[0:ksz, kc, tap, :],
                                wl[0:ksz, kc * 9 * C + tap * C:
                                   kc * 9 * C + tap * C + C],
                                start=(k == 0), stop=False)
                            k += 1
                    for s in range(5):
                        pp = 128 if s < 4 else 64
                        nc.tensor.matmul(
                            ps_t[0:pi_tot, 0:C],
                            rsb2[0:pp, s, :],
                            wl2t[0:pp, s * C:(s + 1) * C],
                            start=False, stop=(s == 4))
                    tsb = work.tile([128, C], fp16, name="tsb", tag="tsb")
                    nc.scalar.copy(tsb[0:pi_tot, :], ps_t[0:pi_tot, 0:C])
                    # masked pair stationary for mc2: pair p cols 0:64 =
                    # tsb mc2 rows of batch 2p (others zeroed), cols 64:128 =
                    # batch 2p+1 — built with diagonal-mask matmuls since
                    # partition offsets must be 32-aligned for direct copies
                    npair = ndeg // 2
                    tsb2 = work.tile([128, max(npair, 1) * 128], fp16,
                                     name="tsb2", tag="tsb2")
                    for p in range(npair):
                        ps_m = psB.tile([128, 512], f32, name=f"psm{p}",
                                        tag="psB")
                        for half in range(2):
                            nc.tensor.matmul(
                                ps_m[0:pi_tot, half * 64:half * 64 + 64],
                                dd[0:pi_tot, p * 144 + half * 72:
                                   p * 144 + half * 72 + 72],
                                tsb[0:pi_tot, 256:320],
                                start=True, stop=True)
                        nc.vector.tensor_copy(tsb2[0:pi_tot,
                                                   p * 128:(p + 1) * 128],
                                              ps_m[0:pi_tot, 0:128])

                # ---------- expansion + dense conv, interleaved per mc ----
                for mc in (0, 2, 1):
                    moff, msz = MB[mc]
                    if ndeg and mc < 2:
                        for bi in range(ndeg):
                            ps_e = psB.tile([128, 512], f32,
                                            name=f"pse{mc}_{bi}", tag="psB")
                            nc.tensor.matmul(
                                ps_e[0:msz, :],
                                tsb[0:pi_tot, moff:moff + msz],
                                e[0:pi_tot, bi * SW:(bi + 1) * SW],
                                start=True, stop=True)
                            dst = ob01[0:msz, bi, mc, :]
                            if bi % 2 == 0:
                                nc.vector.tensor_scalar(
                                    dst, ps_e[0:msz, :],
                                    bias_t[0:msz, mc:mc + 1], 0.0,
                                    mybir.AluOpType.add,
                                    mybir.AluOpType.max)
                            else:
                                nc.scalar.activation(
                                    dst, ps_e[0:msz, :],
                                    mybir.ActivationFunctionType.Relu,
                                    bias=bias_t[0:msz, mc:mc + 1])
                    elif ndeg:
                        for p in range(npair):
                            ps_e = psB.tile([128, 512], f32,
                                            name=f"pse2_{p}", tag="psB")
                            nc.tensor.matmul(
                                ps_e[0:128, :],
                                tsb2[0:pi_tot, p * 128:(p + 1) * 128],
                                e2[0:pi_tot, p * SW:(p + 1) * SW],
                                start=True, stop=True)
                            nc.vector.tensor_scalar(
                                ob2[0:64, 2 * p, :], ps_e[0:64, :],
                                bias_t[0:64, 2:3], 0.0,
                                mybir.AluOpType.add, mybir.AluOpType.max)
                            nc.scalar.activation(
                                ob2[0:64, 2 * p + 1, :], ps_e[64:128, :],
                                mybir.ActivationFunctionType.Relu,
                                bias=bias_t[0:64, 2:3])
                        if ndeg % 2:
                            bi = ndeg - 1
                            ps_e = psB.tile([128, 512], f32,
                                            name="pse2_last", tag="psB")
                            nc.tensor.matmul(
                                ps_e[0:msz, :],
                                tsb[0:pi_tot, moff:moff + msz],
                                e[0:pi_tot, bi * SW:(bi + 1) * SW],
                                start=True, stop=True)
                            nc.vector.tensor_scalar(
                                ob2[0:msz, bi, :], ps_e[0:msz, :],
                                bias_t[0:msz, 2:3], 0.0,
                                mybir.AluOpType.add, mybir.AluOpType.max)
                    for di in range(ndense):
                        smp = smps[di]
                        ps_c = psB.tile([128, 512], f32,
                                        name=f"psc{di}_{mc}", tag="psB")
                        # build the 23 contraction chunks as (stat, mov)
                        chunks = []
                        for kc, (koff, ksz) in enumerate(MB[:2]):
                            for tap in range(9):
                                dy, dx = tap // 3, tap % 3
                                chunks.append((
                                    wl[0:ksz,
                                       kc * 9 * C + tap * C + moff:
                                       kc * 9 * C + tap * C + moff + msz],
                                    smp[0:ksz, kc, dy:dy + 8, dx:dx + 64]))
                        for dx in range(3):
                            chunks.append((
                                wl2[0:128, dx * C + moff:
                                    dx * C + moff + msz],
                                smp[0:128, 2, 0:8, dx:dx + 64]))
                        chunks.append((wl2x[0:128, moff:moff + msz],
                                       smp[0:128, 3, 0:8, 0:64]))
                        chunks.append((
                            wl[0:64, 2 * 9 * C + 8 * C + moff:
                               2 * 9 * C + 8 * C + moff + msz],
                            smp[0:64, 2, 2:10, 2:66]))
                        if mc < 2:
                            # full-width output: single accumulation chain
                            for k, (st, mv) in enumerate(chunks):
                                nc.tensor.matmul(
                                    ps_c[0:msz, :], st, mv,
                                    start=(k == 0), stop=(k == 22))
                            dst = ob01[0:msz, ndeg + di, mc, :]
                            if mc == 1:
                                nc.vector.tensor_scalar(
                                    dst, ps_c[0:msz, :],
                                    bias_t[0:msz, mc:mc + 1], 0.0,
                                    mybir.AluOpType.add, mybir.AluOpType.max)
                            else:
                                nc.scalar.activation(
                                    dst, ps_c[0:msz, :],
                                    mybir.ActivationFunctionType.Relu,
                                    bias=bias_t[0:msz, mc:mc + 1])
                        else:
                            # 64-wide output: two half-chains in opposite PE
                            # column quadrants overlap ~2x; partials summed
                            # at drain
                            ps_cb = psB.tile([128, 512], f32,
                                             name=f"pscb{di}", tag="psB")
                            ha, hb = chunks[0:12], chunks[12:23]
                            for k in range(12):
                                st, mv = ha[k]
                                nc.tensor.matmul(
                                    ps_c[0:64, :], st, mv,
                                    start=(k == 0), stop=(k == 11),
                                    tile_position=(0, 0))
                                if k < len(hb):
                                    st, mv = hb[k]
                                    nc.tensor.matmul(
                                        ps_cb[64:128, :], st, mv,
                                        start=(k == 0),
                                        stop=(k == len(hb) - 1),
                                        tile_position=(0, 64))
                            dst = ob2[0:msz, ndeg + di, :]
                            cv2 = work.tile([64, 512], fp16,
                                            name=f"cv2{di}", tag=f"cv2{di}")
                            nc.scalar.copy(cv2[:, :], ps_cb[64:128, :])
                            nc.vector.scalar_tensor_tensor(
                                dst, ps_c[0:64, :],
                                bias_t[0:64, 2:3], cv2[:, :],
                                mybir.AluOpType.add, mybir.AluOpType.add)
                            nc.gpsimd.tensor_scalar_max(dst, dst, 0.0)
                # ---------- output DMAs ----------
                # deg slices complete after the mc2 expansion drains and
                # ship while the last conv block still runs
                nc.sync.dma_start(out=ob01_d[:, 0:ndeg * 2 * SW],
                                  in_=ob01[:, 0:ndeg, :, :])
                nc.sync.dma_start(out=ob2_d[:, 0:ndeg * SW],
                                  in_=ob2[:, 0:ndeg, :])
                nc.sync.dma_start(out=ob01_d[:, ndeg * 2 * SW:],
                                  in_=ob01[:, ndeg:, :, :])
                nc.sync.dma_start(out=ob2_d[:, ndeg * SW:],
                                  in_=ob2[:, ndeg:, :])

            if reps < 0:
                # unrolled, no hardware loop (TimelineSim profiling)
                for u in range(-reps):
                    body(u)
            elif reps == 1:
                body(0)
            else:
                U = 8
                n_loop = reps // U
                hints = (mybir.EngineType.PE, mybir.EngineType.Activation,
                         mybir.EngineType.Pool, mybir.EngineType.SP,
                         mybir.EngineType.DVE)
                with tc.For_i(0, n_loop, 1, hint_engines=hints) as it:
                    for u in range(U):
                        body(u)
                for u in range(reps - n_loop * U):
                    body(u)

    nc.finalize()
    _NC_CACHE[key] = nc
    return nc


# ---------------------------------------------------------------- interface


def make_in_maps(x, source_intrinsics, target_intrinsics, source_pose,
                 target_pose, conv_w, conv_b):
    return make_plan(x, source_intrinsics, target_intrinsics, source_pose,
                     target_pose, conv_w, conv_b)


def assemble(results, slots):
    """results: per-core {"ob01": [128, NB*2*SW], "ob2": [64, NB*SW]}."""
    out = np.zeros((B, C, H, W), dtype=np.float32)
    NBl = len(slots)
    for r in range(NCORE):
        o01 = np.asarray(results[r]["ob01"]).reshape(128, NBl, 2, STRIP, W)
        o2 = np.asarray(results[r]["ob2"]).reshape(64, NBl, STRIP, W)
        for si, gb in enumerate(slots):
            out[gb, 0:128, 8 * r: 8 * r + STRIP, :] = \
                o01[:, si, 0].astype(np.float32)
            out[gb, 128:256, 8 * r: 8 * r + STRIP, :] = \
                o01[:, si, 1].astype(np.float32)
            out[gb, 256:320, 8 * r: 8 * r + STRIP, :] = \
                o2[:, si].astype(np.float32)
    return out


def kernel(x, source_intrinsics, target_intrinsics, source_pose,
           target_pose, conv_w, conv_b, _reps=1):
    from concourse.bass_utils import run_bass_kernel_spmd
    in_maps, struct, slots = make_in_maps(
        x, source_intrinsics, target_intrinsics, source_pose,
        target_pose, conv_w, conv_b)
    nc = build_program(_reps, struct)
    res = run_bass_kernel_spmd(nc, in_maps, list(range(NCORE)))
    return assemble(res.results, slots)


# revision 38
# speedup vs baseline: 1.0376x; 1.0032x over previous
"""Trainium2 Bass kernel for nn_EpipolarWarpOperator (B=8, C=320, H=W=64, S=3).

Sharding: spatial — every core computes an 8-row horizontal strip of all 8
batch outputs (the 3x3 conv is strip-separable; epipolar sampling per strip
with a 1-row halo).

Host analysis classifies each batch by its epipolar sampling map:

* pattern batches: the per-pixel bilinear sampling signature map has few
  distinct values, so the sampled image is piecewise constant and each output
  column is one of npat distinct "pattern" columns. The device computes
  R[k, tap, pi] from a host-gathered slab via tiny matmuls (kc=2 taps stored
  pairwise over the partition dim), T^T[pi, ch] = sum R_tap^T W_tap (23
  matmuls), then expands out = T^T.T @ E with a per-core 0/1 selection
  matrix E, bias+relu fused into the PSUM drains.

* dense batches: the whole image touches only a few hundred distinct bilinear
  corner pixels; the host gathers them into per-window slabs [128, C] plus
  window sampling matrices (3 windows: 4+4+2 rows covering the strip plus a
  1-row conv halo); the device runs swapped-operand matmuls accumulating the
  slabs into PSUM, then a 24-matmul-per-mc 3x3 conv over the 10-row sampled
  plane (kc=2 taps paired through a +1-row shifted duplicate) with bias+relu
  on the drain.
"""

import numpy as np

B, C, H, W = 8, 320, 64, 64
SN = 3
HW = H * W
STRIP = 8             # output rows per core
NCORE = 8
MB = [(0, 128), (128, 128), (256, 64)]   # output/input channel chunking
PI_MAX = 126          # total pattern budget
NPAT_MAX = 56         # per-batch pattern cap
NSIG_MAX = 64
SW = STRIP * W        # 512 px per strip
WINS = ((-1, 3), (2, 3), (5, 4))   # dense sampling windows (row0, nrows)

# ---------------------------------------------------------------- host: geometry


def _rodrigues_np(rv):
    theta = np.sqrt((rv * rv).sum())
    r = rv / max(theta, 1e-12)
    I = np.eye(3, dtype=np.float32)
    K = np.array([[0, -r[2], r[1]], [r[2], 0, -r[0]], [-r[1], r[0], 0]],
                 dtype=np.float32)
    R = np.cos(theta) * I + (1 - np.cos(theta)) * np.outer(r, r) + np.sin(theta) * K
    return I if theta < 1e-6 else R


def fundamental_np(Ks, Kt, ps, pt):
    Fs = []
    for b in range(Ks.shape[0]):
        Rs = _rodrigues_np(ps[b, :3].astype(np.float32))
        Rt = _rodrigues_np(pt[b, :3].astype(np.float32))
        ts_, tt_ = ps[b, 3:].astype(np.float32), pt[b, 3:].astype(np.float32)
        R_rel = Rs @ Rt.T
        t_rel = ts_ - R_rel @ tt_
        z = np.float32(0)
        skew = np.array([[z, -t_rel[2], t_rel[1]],
                         [t_rel[2], z, -t_rel[0]],
                         [-t_rel[1], t_rel[0], z]], dtype=np.float32)
        E = skew @ R_rel
        inv_Ks = np.linalg.inv(Ks[b].astype(np.float32))
        inv_Kt = np.linalg.inv(Kt[b].astype(np.float32))
        Fs.append(inv_Kt.T @ E @ inv_Ks)
    return np.stack(Fs).astype(np.float32)


def geometry(F):
    k = np.arange(HW)
    px = (k % W).astype(np.float32)
    py = (k // W).astype(np.float32)
    P = np.stack([px, py, np.ones_like(px)])
    lines = F.T.astype(np.float32) @ P
    a, b_, c = lines[0], lines[1], lines[2]
    W1, H1 = np.float32(W - 1), np.float32(H - 1)
    EPS = np.float32(1e-10)
    x1 = np.clip(-c / (a + EPS), 0.0, W1)
    x2 = np.clip(-(b_ * H1 + c) / (a + EPS), 0.0, W1)
    y1 = np.clip(-c / (b_ + EPS), 0.0, H1)
    y2 = np.clip(-(a * W1 + c) / (b_ + EPS), 0.0, H1)
    t = np.linspace(0.0, 1.0, SN, dtype=np.float32)
    sx = x1[:, None] * (1 - t) + x2[:, None] * t
    sy = y1[:, None] * (1 - t) + y2[:, None] * t
    x0 = np.floor(sx)
    y0 = np.floor(sy)
    wx = (sx - x0).astype(np.float32)
    wy = (sy - y0).astype(np.float32)
    x0i = np.clip(x0, 0, W - 1).astype(np.int32)
    y0i = np.clip(y0, 0, H - 1).astype(np.int32)
    return x0i, y0i, wx, wy


def _corners(geo, p, s):
    """4 bilinear (source pixel row, weight/3) pairs for pixel p, sample s."""
    x0i, y0i, wx, wy = geo
    y0 = int(y0i[p, s]); x0 = int(x0i[p, s])
    x1 = min(x0 + 1, W - 1); y1 = min(y0 + 1, H - 1)
    wxx = np.float32(wx[p, s]); wyy = np.float32(wy[p, s])
    third = np.float32(1.0 / 3.0)
    out = {}
    for ry, rx, ww in ((y0, x0, (1 - wxx) * (1 - wyy)),
                       (y0, x1, wxx * (1 - wyy)),
                       (y1, x0, (1 - wxx) * wyy),
                       (y1, x1, wxx * wyy)):
        rr = ry * W + rx
        out[rr] = out.get(rr, np.float32(0)) + ww * third
    return out


# ------------------------------------------------------------- host: classify


def classify(geo):
    x0i, y0i, wx, wy = geo
    key = np.concatenate([
        y0i.astype(np.float32), x0i.astype(np.float32), wx, wy], axis=1)
    kview = np.ascontiguousarray(key).view([('', key.dtype)] * key.shape[1]).ravel()
    uniq, first, inv = np.unique(kview, return_index=True, return_inverse=True)
    if len(uniq) > NSIG_MAX:
        return None
    tables = []
    for si in range(len(uniq)):
        p = int(first[si])
        contrib = {}
        for s in range(SN):
            for rr, ww in _corners(geo, p, s).items():
                contrib[rr] = contrib.get(rr, np.float32(0)) + ww
        tables.append(sorted(contrib.items()))
    return inv.astype(np.int32), tables


def patterns(siginv):
    """3x3 neighborhood patterns of the signature map (border = -1)."""
    simg = siginv.reshape(H, W)
    pad = np.pad(simg, 1, constant_values=-1)
    neigh = np.stack([pad[dy:dy + H, dx:dx + W].ravel()
                      for dy in range(3) for dx in range(3)], axis=1)
    uniq, pinv = np.unique(neigh, axis=0, return_inverse=True)
    return pinv.astype(np.int32), uniq


# ------------------------------------------------------------ host: build plan


def _deg_slabs(entries):
    """Assign signatures to slabs so each slab's source-pixel union <= 128."""
    slabs = []
    rows, sigs = {}, []
    for bloc, si, tab in entries:
        new = [(bloc, rr) for rr, _ in tab if (bloc, rr) not in rows]
        if len(rows) + len(new) > 128:
            slabs.append((rows, sigs))
            rows, sigs = {}, []
            new = [(bloc, rr) for rr, _ in tab]
        for key in new:
            rows[key] = len(rows)
        sigs.append((bloc, si))
    slabs.append((rows, sigs))
    return slabs


def _dense_sources(geo, r0, r1):
    """Sorted unique corner source rows for dest rows [r0, r1) (clipped)."""
    x0i, y0i = geo[0], geo[1]
    s = set()
    for r in range(max(r0, 0), min(r1, H)):
        for cx in range(W):
            p = r * W + cx
            for sm in range(SN):
                y0, x0 = int(y0i[p, sm]), int(x0i[p, sm])
                x1, y1 = min(x0 + 1, W - 1), min(y0 + 1, H - 1)
                s.add(y0 * W + x0); s.add(y0 * W + x1)
                s.add(y1 * W + x0); s.add(y1 * W + x1)
    return sorted(s)


def make_plan(x, source_intrinsics, target_intrinsics, source_pose,
              target_pose, conv_w, conv_b):
    Fs = fundamental_np(np.asarray(source_intrinsics, np.float32),
                        np.asarray(target_intrinsics, np.float32),
                        np.asarray(source_pose, np.float32),
                        np.asarray(target_pose, np.float32))
    x = np.asarray(x, np.float32)
    xT16 = [np.ascontiguousarray(x[b].reshape(C, HW).T).astype(np.float16)
            for b in range(B)]

    degs, denses = [], []
    pi_used = 0
    for b in range(B):
        geo = geometry(Fs[b])
        res = classify(geo)
        if res is not None:
            siginv, tables = res
            pinv, ptab = patterns(siginv)
            npat = ptab.shape[0]
            if npat <= NPAT_MAX and pi_used + npat <= PI_MAX:
                degs.append(dict(gb=b, siginv=siginv, tables=tables,
                                 pinv=pinv, ptab=ptab, pi_off=pi_used))
                pi_used += npat
                continue
        denses.append(dict(gb=b, geo=geo))
    pi_tot = pi_used

    # ---- degenerate global data (same for all cores) ----
    RGROUP_CAP = 512
    rgroups = []
    cur, cur_n = [], 0
    for bi, d in enumerate(degs):
        n9 = 9 * d['ptab'].shape[0]
        if cur and cur_n + n9 > RGROUP_CAP:
            rgroups.append(dict(bis=cur, ncols=cur_n))
            cur, cur_n = [], 0
        cur.append(bi)
        cur_n += n9
    if cur:
        rgroups.append(dict(bis=cur, ncols=cur_n))

    sg_blocks, sr_blocks = [], []
    for g in rgroups:
        entries = []
        for bloc, bi in enumerate(g['bis']):
            for si, tab in enumerate(degs[bi]['tables']):
                entries.append((bloc, si, tab))
        slabs = _deg_slabs(entries)
        g['nslab'] = len(slabs)
        g['q0'] = degs[g['bis'][0]]['pi_off']
        g['nq'] = sum(degs[bi]['ptab'].shape[0] for bi in g['bis'])
        nq = g['nq']
        for rows, sigs in slabs:
            slab = np.zeros((128, C), dtype=np.float16)
            for (bloc, rr), idx in rows.items():
                slab[idx] = xT16[degs[g['bis'][bloc]]['gb']][rr]
            SR = np.zeros((128, g['ncols']), dtype=np.float32)
            sigset = set(sigs)
            for bloc, bi in enumerate(g['bis']):
                d = degs[bi]
                ptab = d['ptab']
                qb = d['pi_off'] - g['q0']
                for pi in range(ptab.shape[0]):
                    for tap in range(9):
                        si = ptab[pi, tap]
                        if si >= 0 and (bloc, si) in sigset:
                            for rr, ww in d['tables'][si]:
                                SR[rows[(bloc, rr)],
                                   tap * nq + qb + pi] += ww
            sg_blocks.append(slab)
            sr_blocks.append(SR.astype(np.float16))
    sg = (np.concatenate(sg_blocks, axis=1) if sg_blocks
          else np.zeros((128, 0), np.float16))
    sr = (np.concatenate(sr_blocks, axis=1) if sr_blocks
          else np.zeros((128, 0), np.float16))

    # ---- dense per-core per-window slabs & sampling matrices ----
    core_src = []      # [core][di][win] -> sorted source list
    nslw = [1] * len(WINS)
    for r in range(NCORE):
        per_d = []
        for d in denses:
            per_w = []
            for wi, (w0, wn) in enumerate(WINS):
                sl = _dense_sources(d['geo'], 8 * r + w0, 8 * r + w0 + wn)
                per_w.append(sl)
                nslw[wi] = max(nslw[wi], (len(sl) + 127) // 128)
            per_d.append(per_w)
        core_src.append(per_d)
    NSLW = tuple(nslw)

    # ---- weights ----
    Wl = np.zeros((128, 3 * 9 * C), dtype=np.float16)
    cw = np.asarray(conv_w, np.float32)
    cb = np.asarray(conv_b, np.float32)
    for kc, (koff, ksz) in enumerate(MB):
        for tap in range(9):
            dy, dx = tap // 3, tap % 3
            Wl[0:ksz, kc * 9 * C + tap * C: kc * 9 * C + tap * C + C] = \
                cw[:, koff:koff + ksz, dy, dx].T.astype(np.float16)
    # paired kc=2 weights for the dense conv: partitions 0:64 = dy=0 tap,
    # 64:128 = dy=1 tap (read through the +1-row shifted duplicate plane)
    Wl2 = np.zeros((128, 3 * C), dtype=np.float16)
    for dxi in range(3):
        Wl2[0:64, dxi * C: dxi * C + C] = \
            cw[:, 256:320, 0, dxi].T.astype(np.float16)
        Wl2[64:128, dxi * C: dxi * C + C] = \
            cw[:, 256:320, 1, dxi].T.astype(np.float16)
    # paired kc=2 weights for the T matmul: tap pair (2s, 2s+1) stacked on
    # the partition dim (matches rsb2's paired R layout)
    Wl2t = np.zeros((128, 5 * C), dtype=np.float16)
    for s in range(5):
        t0 = 2 * s
        Wl2t[0:64, s * C:(s + 1) * C] = \
            cw[:, 256:320, t0 // 3, t0 % 3].T.astype(np.float16)
        if t0 + 1 < 9:
            Wl2t[64:128, s * C:(s + 1) * C] = \
                cw[:, 256:320, (t0 + 1) // 3, (t0 + 1) % 3].T.astype(np.float16)
    # paired kc=2 dy=2 weights: partitions 0:64 = dx=0 tap, 64:128 = dx=1
    # (read through the +1-column shifted duplicate plane)
    Wl2x = np.zeros((128, C), dtype=np.float16)
    Wl2x[0:64, :] = cw[:, 256:320, 2, 0].T.astype(np.float16)
    Wl2x[64:128, :] = cw[:, 256:320, 2, 1].T.astype(np.float16)
    bias = np.zeros((128, 3), dtype=np.float32)
    for mc, (moff, msz) in enumerate(MB):
        bias[0:msz, mc] = cb[moff:moff + msz]

    # ---- per-core in_maps ----
    ndeg, ndense = len(degs), len(denses)
    slots = [d['gb'] for d in degs] + [d['gb'] for d in denses]
    in_maps = []
    for r in range(NCORE):
        m = {"wl": Wl, "wl2": Wl2, "wl2t": Wl2t, "wl2x": Wl2x,
             "bias": bias}
        if ndeg:
            m["sg"] = sg
            m["sr"] = sr
            e = np.zeros((128, ndeg * SW), dtype=np.float16)
            for bi, d in enumerate(degs):
                pidx = d['pinv'].reshape(H, W)[8 * r: 8 * r + STRIP].ravel()
                e[d['pi_off'] + pidx,
                  bi * SW + np.arange(SW)] = 1.0
            m["e_mat"] = e
            # mc2 batch-pair overlay: pair (2p, 2p+1) columns summed
            # (disjoint pi rows make the overlay separable by masking)
            npair = ndeg // 2
            e2 = np.zeros((128, (npair + ndeg % 2) * SW), dtype=np.float16)
            for p in range(npair):
                e2[:, p * SW:(p + 1) * SW] = (
                    e[:, 2 * p * SW:(2 * p + 1) * SW]
                    + e[:, (2 * p + 1) * SW:(2 * p + 2) * SW])
            if ndeg % 2:
                e2[:, npair * SW:] = e[:, (ndeg - 1) * SW:ndeg * SW]
            m["e2"] = e2
            pioff = [0]
            for d in degs:
                pioff.append(pioff[-1] + d['ptab'].shape[0])
            dd = np.zeros((128, max(npair, 1) * 144), dtype=np.float16)
            for p in range(npair):
                for half in range(2):
                    bi = 2 * p + half
                    for q in range(pioff[bi], pioff[bi + 1]):
                        dd[q, p * 144 + half * 72 + q] = 1.0
            m["dd"] = dd
        if ndense:
            sds, sss = [], []
            for di, d in enumerate(denses):
                for wi, (w0, wn) in enumerate(WINS):
                    sl = core_src[r][di][wi]
                    rowmap = {rr: i for i, rr in enumerate(sl)}
                    nsl = NSLW[wi]
                    slab = np.zeros((nsl * 128, C), dtype=np.float16)
                    if sl:
                        slab[:len(sl)] = xT16[d['gb']][np.array(sl)]
                    sds.append(slab.reshape(nsl, 128, C).transpose(1, 0, 2)
                               .reshape(128, nsl * C))
                    Smat = np.zeros((nsl * 128, wn * W), dtype=np.float32)
                    for lr in range(wn):
                        row = 8 * r + w0 + lr
                        if not (0 <= row < H):
                            continue
                        for cx in range(W):
                            p = row * W + cx
                            for sm in range(SN):
                                for rr, ww in _corners(d['geo'], p, sm).items():
                                    Smat[rowmap[rr], lr * W + cx] += ww
                    sss.append(Smat.astype(np.float16).reshape(nsl, 128, wn * W)
                               .transpose(1, 0, 2).reshape(128, nsl * wn * W))
            m["sd"] = np.concatenate(sds, axis=1)
            m["ss"] = np.concatenate(sss, axis=1)
        in_maps.append(m)

    struct = (pi_tot, NSLW,
              tuple((d['gb'], d['ptab'].shape[0]) for d in degs),
              tuple((g['ncols'], g['nslab'], g['q0'], g['nq'])
                    for g in rgroups),
              tuple(d['gb'] for d in denses))
    return in_maps, struct, slots


# ------------------------------------------------------------- bass program

_NC_CACHE = {}


def build_program(reps, struct):
    key = (reps, struct)
    if key in _NC_CACHE:
        return _NC_CACHE[key]
    import concourse.bacc as bacc
    import concourse.mybir as mybir
    from concourse.tile import TileContext

    fp16 = mybir.dt.float16
    f32 = mybir.dt.float32
    pi_tot, NSLW, degs, rgroups, dense_gbs = struct
    ndeg, ndense = len(degs), len(dense_gbs)
    NB = ndeg + ndense
    NSG = sum(ns for _, ns, _, _ in rgroups)
    SRC = sum(nc_ * ns for nc_, ns, _, _ in rgroups)
    NSLT = sum(NSLW)                      # total slabs per dense batch
    SSW = sum(NSLW[w] * WINS[w][1] * W for w in range(len(WINS)))

    nc = bacc.Bacc(target_bir_lowering=False)
    wl_d = nc.dram_tensor("wl", [128, 3 * 9 * C], fp16, kind="ExternalInput")
    wl2_d = nc.dram_tensor("wl2", [128, 3 * C], fp16, kind="ExternalInput")
    wl2t_d = nc.dram_tensor("wl2t", [128, 5 * C], fp16, kind="ExternalInput")
    wl2x_d = nc.dram_tensor("wl2x", [128, C], fp16, kind="ExternalInput")
    bias_d = nc.dram_tensor("bias", [128, 3], f32, kind="ExternalInput")
    if ndeg:
        sg_d = nc.dram_tensor("sg", [128, NSG * C], fp16, kind="ExternalInput")
        sr_d = nc.dram_tensor("sr", [128, SRC], fp16, kind="ExternalInput")
        e_d = nc.dram_tensor("e_mat", [128, ndeg * SW], fp16,
                             kind="ExternalInput")
        e2_d = nc.dram_tensor(
            "e2", [128, (ndeg // 2 + ndeg % 2) * SW], fp16,
            kind="ExternalInput")
        dd_d = nc.dram_tensor("dd", [128, max(ndeg // 2, 1) * 144], fp16,
                              kind="ExternalInput")
    if ndense:
        sd_d = nc.dram_tensor("sd", [128, ndense * NSLT * C], fp16,
                              kind="ExternalInput")
        ss_d = nc.dram_tensor("ss", [128, ndense * SSW], fp16,
                              kind="ExternalInput")
    ob01_d = nc.dram_tensor("ob01", [128, NB * 2 * SW], fp16,
                            kind="ExternalOutput")
    ob2_d = nc.dram_tensor("ob2", [64, NB * SW], fp16, kind="ExternalOutput")

    with TileContext(nc) as tc:
        with tc.tile_pool(name="const", bufs=1) as constp, \
             tc.tile_pool(name="inp", bufs=2) as inp, \
             tc.tile_pool(name="sdp", bufs=2) as sdp, \
             tc.tile_pool(name="work", bufs=2) as work, \
             tc.tile_pool(name="smpp", bufs=2) as smpp, \
             tc.tile_pool(name="outp", bufs=3) as outp, \
             tc.tile_pool(name="psA", bufs=3, space="PSUM") as psA, \
             tc.tile_pool(name="psB", bufs=5, space="PSUM") as psB:
            wl = constp.tile([128, 3 * 9 * C], fp16)
            nc.sync.dma_start(out=wl[:], in_=wl_d[:])
            wl2 = constp.tile([128, 3 * C], fp16)
            nc.sync.dma_start(out=wl2[:], in_=wl2_d[:])
            wl2t = constp.tile([128, 5 * C], fp16)
            nc.sync.dma_start(out=wl2t[:], in_=wl2t_d[:])
            wl2x = constp.tile([128, C], fp16)
            nc.sync.dma_start(out=wl2x[:], in_=wl2x_d[:])
            bias_t = constp.tile([128, 3], f32)
            nc.sync.dma_start(out=bias_t[:], in_=bias_d[:])

            def body(_it):
                ob01 = outp.tile([128, NB, 2, SW], fp16, name="ob01",
                                 tag="ob01")
                ob2 = outp.tile([64, NB, SW], fp16, name="ob2", tag="ob2")

                # ---------- input DMAs ----------
                if ndeg:
                    sg = inp.tile([128, NSG * C], fp16, name="sg", tag="sg")
                    nc.sync.dma_start(out=sg[:], in_=sg_d[:])
                    sr = inp.tile([128, SRC], fp16, name="sr", tag="sr")
                    nc.sync.dma_start(out=sr[:], in_=sr_d[:])
                if ndense:
                    sd = sdp.tile([128, ndense * NSLT * C], fp16, name="sd",
                                  tag="sd")
                    nc.sync.dma_start(out=sd[:], in_=sd_d[:])
                    ss = sdp.tile([128, ndense * SSW], fp16, name="ss",
                                  tag="ss")
                    nc.sync.dma_start(out=ss[:], in_=ss_d[:])
                if ndeg:
                    # needed only by the late expansion matmuls
                    e = inp.tile([128, ndeg * SW], fp16, name="e", tag="e")
                    nc.sync.dma_start(out=e[0:pi_tot, :],
                                      in_=e_d[0:pi_tot, :])
                    e2 = inp.tile([128, (ndeg // 2 + ndeg % 2) * SW], fp16,
                                  name="e2", tag="e2")
                    nc.sync.dma_start(out=e2[0:pi_tot, :],
                                      in_=e2_d[0:pi_tot, :])
                    dd = inp.tile([128, max(ndeg // 2, 1) * 144], fp16,
                                  name="dd", tag="dd")
                    nc.sync.dma_start(out=dd[0:pi_tot, :],
                                      in_=dd_d[0:pi_tot, :])

                # ---------- degenerate path: R ----------
                if ndeg:
                    # kc0/kc1 R in [ksz, kc, tap, q]; kc2 R pairwise:
                    # rsb2[0:64, s, q] = tap 2s, rsb2[64:128, s, q] = tap 2s+1
                    rsb = work.tile([128, 2, 9, pi_tot], fp16, name="rsb",
                                    tag="rsb")
                    rsb2 = work.tile([128, 5, pi_tot], fp16, name="rsb2",
                                     tag="rsb2")
                    sgo, sro = 0, 0
                    for gi, (ncols, nsl, q0, nq) in enumerate(rgroups):
                        for kc, (koff, ksz) in enumerate(MB):
                            ps_r = psB.tile([128, 512], f32,
                                            name=f"psr{gi}_{kc}", tag="psB")
                            for j in range(nsl):
                                nc.tensor.matmul(
                                    ps_r[0:ksz, 0:ncols],
                                    sg[:, (sgo + j) * C + koff:
                                       (sgo + j) * C + koff + ksz],
                                    sr[:, sro + j * ncols:
                                       sro + (j + 1) * ncols],
                                    start=(j == 0), stop=(j == nsl - 1))
                            psq = ps_r[:, 0:ncols].rearrange(
                                "p (t q) -> p t q", t=9)
                            if kc < 2:
                                eng = nc.vector.tensor_copy if kc == 0 \
                                    else nc.scalar.copy
                                eng(rsb[0:ksz, kc, :, q0:q0 + nq], psq[0:ksz])
                            else:
                                nc.vector.tensor_copy(
                                    rsb2[0:64, :, q0:q0 + nq],
                                    psq[0:64, 0:9:2])
                                nc.scalar.copy(
                                    rsb2[64:128, 0:4, q0:q0 + nq],
                                    psq[0:64, 1:9:2])
                        sgo += nsl
                        sro += nsl * ncols

                # ---------- dense sampling: 3 windows ----------
                smps = []
                if ndense:
                    for di in range(ndense):
                        # plane row i = image row 8r-1+i; cols 1..64 = image
                        smp = smpp.tile([128, 4, 10, 66], fp16,
                                        name=f"smp{di}", tag=f"smp{di}")
                        nc.gpsimd.memset(smp[:, :, :, 0:1], 0.0)
                        nc.gpsimd.memset(smp[:, :, :, 65:66], 0.0)
                        nc.gpsimd.memset(smp[64:128, 3, 0:8, 64:65], 0.0)
                        smps.append(smp)

                    for di in range(ndense):
                        smp = smps[di]
                        sdo = di * NSLT * C
                        sso = di * SSW
                        # kc2 (64-wide out) of windows 0/1 runs as two
                        # interleaved accumulation chains in opposite PE
                        # column quadrants (~2x overlap); window 2 inline
                        k2t = [psA.tile([128, 512], f32, name=f"pk2{di}_{i}",
                                        tag="psA") for i in range(3)]
                        k2mm = [[], [], []]
                        sdo0, sso0 = sdo, sso
                        for wi, (w0, wn) in enumerate(WINS):
                            px = wn * W
                            nsl = NSLW[wi]
                            pw = psA.tile([128, 512], f32,
                                          name=f"psw{di}_{wi}", tag="psA")
                            qrow = 64 if wi == 1 else 0
                            dsts = [pw[0:128, 0:px], pw[0:128, px:2 * px],
                                    k2t[wi][qrow:qrow + 64, 0:px]]
                            for kc, (koff, ksz) in enumerate(MB):
                                for j in range(nsl):
                                    args = (
                                        dsts[kc][0:ksz, :],
                                        sd[:, sdo + j * C + koff:
                                           sdo + j * C + koff + ksz],
                                        ss[:, sso + j * px:sso + (j + 1) * px])
                                    if kc < 2:
                                        nc.tensor.matmul(
                                            *args, start=(j == 0),
                                            stop=(j == nsl - 1))
                                    else:
                                        k2mm[wi].append(
                                            (args, j == 0, j == nsl - 1))
                            r0 = w0 + 1   # plane row of window start
                            pv = pw[:, 0:2 * px].rearrange(
                                "p (k r c) -> p k r c", k=2, r=wn)
                            nc.vector.tensor_copy(
                                smp[0:128, 0:2, r0:r0 + wn, 1:65],
                                pv[0:128])
                            sdo += nsl * C
                            sso += nsl * px
                        # interleave win0/win1 kc2 chains, then win2
                        for j in range(max(len(k2mm[0]), len(k2mm[1]))):
                            for wi in (0, 1):
                                if j < len(k2mm[wi]):
                                    args, st_, sp_ = k2mm[wi][j]
                                    nc.tensor.matmul(
                                        *args, start=st_, stop=sp_,
                                        tile_position=(0, 64 if wi else 0))
                        for args, st_, sp_ in k2mm[2]:
                            nc.tensor.matmul(*args, start=st_, stop=sp_)
                        for wi, (w0, wn) in enumerate(WINS):
                            px = wn * W
                            r0 = w0 + 1
                            qrow = 64 if wi == 1 else 0
                            nc.scalar.copy(
                                smp[0:64, 2, r0:r0 + wn, 1:65],
                                k2t[wi][qrow:qrow + 64, 0:px].rearrange(
                                    "p (r c) -> p r c", r=wn))
                        # +1-row shifted duplicate of kc=2 plane in parts
                        # 64:128 (for the paired dy0/dy1 conv taps)
                        nc.gpsimd.tensor_copy(smp[64:128, 2, 0:9, 1:65],
                                              smp[0:64, 2, 1:10, 1:65])
                        # plane 3: kc2 rows 2:10; parts 64:128 shifted +1 col
                        # (for the paired dy=2 dx=0/dx=1 conv taps)
                        nc.gpsimd.tensor_copy(smp[0:64, 3, 0:8, 1:65],
                                              smp[0:64, 2, 2:10, 1:65])
                        nc.gpsimd.tensor_copy(smp[64:128, 3, 0:8, 0:65],
                                              smp[0:64, 2, 2:10, 1:66])

                # ---------- degenerate path: T ----------
                if ndeg:
                    ps_t = psB.tile([128, 512], f32, name="ps_t", tag="psB")
                    k = 0
                    for kc, (koff, ksz) in enumerate(MB[:2]):
                        for tap in range(9):
                            nc.tensor.matmul(
                                ps_t[0:pi_tot, 0:C],
                                rsb[0:ksz, kc, tap, :],
                                wl[0:ksz, kc * 9 * C + tap * C:
                                   kc * 9 * C + tap * C + C],
                                start=(k == 0), stop=False)
                            k += 1
                    for s in range(5):
                        pp = 128 if s < 4 else 64
                        nc.tensor.matmul(
                            ps_t[0:pi_tot, 0:C],
                            rsb2[0:pp, s, :],
                            wl2t[0:pp, s * C:(s + 1) * C],
                            start=False, stop=(s == 4))
                    tsb = work.tile([128, C], fp16, name="tsb", tag="tsb")
                    nc.scalar.copy(tsb[0:pi_tot, :], ps_t[0:pi_tot, 0:C])
                    # masked pair stationary for mc2: pair p cols 0:64 =
                    # tsb mc2 rows of batch 2p (others zeroed), cols 64:128 =
                    # batch 2p+1 — built with diagonal-mask matmuls since
                    # partition offsets must be 32-aligned for direct copies
                    npair = ndeg // 2
                    tsb2 = work.tile([128, max(npair, 1) * 128], fp16,
                                     name="tsb2", tag="tsb2")
                    for p in range(npair):
                        ps_m = psB.tile([128, 512], f32, name=f"psm{p}",
                                        tag="psB")
                        for half in range(2):
                            nc.tensor.matmul(
                                ps_m[0:pi_tot, half * 64:half * 64 + 64],
                                dd[0:pi_tot, p * 144 + half * 72:
                                   p * 144 + half * 72 + 72],
                                tsb[0:pi_tot, 256:320],
                                start=True, stop=True)
                        nc.vector.tensor_copy(tsb2[0:pi_tot,
                                                   p * 128:(p + 1) * 128],
                                              ps_m[0:pi_tot, 0:128])

                # ---------- expansion + dense conv, interleaved per mc ----
                for mc in (2, 0, 1):
                    moff, msz = MB[mc]
                    if ndeg and mc < 2:
                        for bi in range(ndeg):
                            ps_e = psB.tile([128, 512], f32,
                                            name=f"pse{mc}_{bi}", tag="psB")
                            nc.tensor.matmul(
                                ps_e[0:msz, :],
                                tsb[0:pi_tot, moff:moff + msz],
                                e[0:pi_tot, bi * SW:(bi + 1) * SW],
                                start=True, stop=True)
                            dst = ob01[0:msz, bi, mc, :]
                            if bi % 2 == 0:
                                nc.vector.tensor_scalar(
                                    dst, ps_e[0:msz, :],
                                    bias_t[0:msz, mc:mc + 1], 0.0,
                                    mybir.AluOpType.add,
                                    mybir.AluOpType.max)
                            else:
                                nc.scalar.activation(
                                    dst, ps_e[0:msz, :],
                                    mybir.ActivationFunctionType.Relu,
                                    bias=bias_t[0:msz, mc:mc + 1])
                    elif ndeg:
                        for p in range(npair):
                            ps_e = psB.tile([128, 512], f32,
                                            name=f"pse2_{p}", tag="psB")
                            nc.tensor.matmul(
                                ps_e[0:128, :],
                                tsb2[0:pi_tot, p * 128:(p + 1) * 128],
                                e2[0:pi_tot, p * SW:(p + 1) * SW],
                                start=True, stop=True)
                            nc.vector.tensor_scalar(
                                ob2[0:64, 2 * p, :], ps_e[0:64, :],
                                bias_t[0:64, 2:3], 0.0,
                                mybir.AluOpType.add, mybir.AluOpType.max)
                            nc.scalar.activation(
                                ob2[0:64, 2 * p + 1, :], ps_e[64:128, :],
                                mybir.ActivationFunctionType.Relu,
                                bias=bias_t[0:64, 2:3])
                        if ndeg % 2:
                            bi = ndeg - 1
                            ps_e = psB.tile([128, 512], f32,
                                            name="pse2_last", tag="psB")
                            nc.tensor.matmul(
                                ps_e[0:msz, :],
                                tsb[0:pi_tot, moff:moff + msz],
                                e[0:pi_tot, bi * SW:(bi + 1) * SW],
                                start=True, stop=True)
                            nc.vector.tensor_scalar(
                                ob2[0:msz, bi, :], ps_e[0:msz, :],
                                bias_t[0:msz, 2:3], 0.0,
                                mybir.AluOpType.add, mybir.AluOpType.max)
                    for di in range(ndense):
                        smp = smps[di]
                        ps_c = psB.tile([128, 512], f32,
                                        name=f"psc{di}_{mc}", tag="psB")
                        # build the 23 contraction chunks as (stat, mov)
                        chunks = []
                        for kc, (koff, ksz) in enumerate(MB[:2]):
                            for tap in range(9):
                                dy, dx = tap // 3, tap % 3
                                chunks.append((
                                    wl[0:ksz,
                                       kc * 9 * C + tap * C + moff:
                                       kc * 9 * C + tap * C + moff + msz],
                                    smp[0:ksz, kc, dy:dy + 8, dx:dx + 64]))
                        for dx in range(3):
                            chunks.append((
                                wl2[0:128, dx * C + moff:
                                    dx * C + moff + msz],
                                smp[0:128, 2, 0:8, dx:dx + 64]))
                        chunks.append((wl2x[0:128, moff:moff + msz],
                                       smp[0:128, 3, 0:8, 0:64]))
                        chunks.append((
                            wl[0:64, 2 * 9 * C + 8 * C + moff:
                               2 * 9 * C + 8 * C + moff + msz],
                            smp[0:64, 2, 2:10, 2:66]))
                        if mc < 2:
                            # full-width output: single accumulation chain
                            for k, (st, mv) in enumerate(chunks):
                                nc.tensor.matmul(
                                    ps_c[0:msz, :], st, mv,
                                    start=(k == 0), stop=(k == 22))
                            dst = ob01[0:msz, ndeg + di, mc, :]
                            if mc == 1:
                                nc.vector.tensor_scalar(
                                    dst, ps_c[0:msz, :],
                                    bias_t[0:msz, mc:mc + 1], 0.0,
                                    mybir.AluOpType.add, mybir.AluOpType.max)
                            else:
                                nc.scalar.activation(
                                    dst, ps_c[0:msz, :],
                                    mybir.ActivationFunctionType.Relu,
                                    bias=bias_t[0:msz, mc:mc + 1])
                        else:
                            # 64-wide output: two half-chains in opposite PE
                            # column quadrants overlap ~2x; partials summed
                            # at drain
                            ps_cb = psB.tile([128, 512], f32,
                                             name=f"pscb{di}", tag="psB")
                            ha, hb = chunks[0:12], chunks[12:23]
                            for k in range(12):
                                st, mv = ha[k]
                                nc.tensor.matmul(
                                    ps_c[0:64, :], st, mv,
                                    start=(k == 0), stop=(k == 11),
                                    tile_position=(0, 0))
                                if k < len(hb):
                                    st, mv = hb[k]
                                    nc.tensor.matmul(
                                        ps_cb[64:128, :], st, mv,
                                        start=(k == 0),
                                        stop=(k == len(hb) - 1),
                                        tile_position=(0, 64))
                            dst = ob2[0:msz, ndeg + di, :]
                            cv2 = work.tile([64, 512], fp16,
                                            name=f"cv2{di}", tag=f"cv2{di}")
                            nc.scalar.copy(cv2[:, :], ps_cb[64:128, :])
                            nc.vector.scalar_tensor_tensor(
                                dst, ps_c[0:64, :],
                                bias_t[0:64, 2:3], cv2[:, :],
                                mybir.AluOpType.add, mybir.AluOpType.add)
                            nc.gpsimd.tensor_scalar_max(dst, dst, 0.0)
                # ---------- output DMAs ----------
                # deg slices complete after the mc2 expansion drains and
                # ship while the last conv block still runs
                nc.sync.dma_start(out=ob01_d[:, 0:ndeg * 2 * SW],
                                  in_=ob01[:, 0:ndeg, :, :])
                nc.sync.dma_start(out=ob2_d[:, 0:ndeg * SW],
                                  in_=ob2[:, 0:ndeg, :])
                nc.sync.dma_start(out=ob01_d[:, ndeg * 2 * SW:],
                                  in_=ob01[:, ndeg:, :, :])
                nc.sync.dma_start(out=ob2_d[:, ndeg * SW:],
                                  in_=ob2[:, ndeg:, :])

            if reps < 0:
                # unrolled, no hardware loop (TimelineSim profiling)
                for u in range(-reps):
                    body(u)
            elif reps == 1:
                body(0)
            else:
                U = 8
                n_loop = reps // U
                hints = (mybir.EngineType.PE, mybir.EngineType.Activation,
                         mybir.EngineType.Pool, mybir.EngineType.SP,
                         mybir.EngineType.DVE)
                with tc.For_i(0, n_loop, 1, hint_engines=hints) as it:
                    for u in range(U):
                        body(u)
                for u in range(reps - n_loop * U):
                    body(u)

    nc.finalize()
    _NC_CACHE[key] = nc
    return nc


# ---------------------------------------------------------------- interface


def make_in_maps(x, source_intrinsics, target_intrinsics, source_pose,
                 target_pose, conv_w, conv_b):
    return make_plan(x, source_intrinsics, target_intrinsics, source_pose,
                     target_pose, conv_w, conv_b)


def assemble(results, slots):
    """results: per-core {"ob01": [128, NB*2*SW], "ob2": [64, NB*SW]}."""
    out = np.zeros((B, C, H, W), dtype=np.float32)
    NBl = len(slots)
    for r in range(NCORE):
        o01 = np.asarray(results[r]["ob01"]).reshape(128, NBl, 2, STRIP, W)
        o2 = np.asarray(results[r]["ob2"]).reshape(64, NBl, STRIP, W)
        for si, gb in enumerate(slots):
            out[gb, 0:128, 8 * r: 8 * r + STRIP, :] = \
                o01[:, si, 0].astype(np.float32)
            out[gb, 128:256, 8 * r: 8 * r + STRIP, :] = \
                o01[:, si, 1].astype(np.float32)
            out[gb, 256:320, 8 * r: 8 * r + STRIP, :] = \
                o2[:, si].astype(np.float32)
    return out


def kernel(x, source_intrinsics, target_intrinsics, source_pose,
           target_pose, conv_w, conv_b, _reps=1):
    from concourse.bass_utils import run_bass_kernel_spmd
    in_maps, struct, slots = make_in_maps(
        x, source_intrinsics, target_intrinsics, source_pose,
        target_pose, conv_w, conv_b)
    nc = build_program(_reps, struct)
    res = run_bass_kernel_spmd(nc, in_maps, list(range(NCORE)))
    return assemble(res.results, slots)
